# revision 1
# baseline (speedup 1.0000x reference)
"""GQA attention block (RMSNorm-QK, causal, GQA) on 8 trn2 NeuronCores.

Strategy: sequence sharding, zero collectives. Core c handles batch c//4 and
two causally-balanced query chunks (j and 7-j of 8) of 256 rows each. The host
permutes the key/token axis per core so every core sees its own query tokens
at fixed columns [0:512]; causality becomes per-core *data* (exp-bias columns
+ two constant triangle masks), so one uniform SPMD program serves all cores.

All activations live feature-major ("T layout", [feature, token]) so every
matmul consumes natural layouts with zero on-device transposes except V
(tiny). Scores are computed transposed ([k, q]); softmax needs no max
subtraction because RMS-normalized q,k bound |scores/sqrt(D)| <= sqrt(D).
Partition-dim reductions (RMS sum-of-squares, softmax denominators) are
rank-1 matmuls on the PE; per-token broadcasts are rank-1 matmuls as well.
"""

import math
import numpy as np
from contextlib import ExitStack

import concourse.bass as bass
import concourse.mybir as mybir
import concourse.tile as tile
from concourse import bacc
from concourse.bass_utils import run_bass_kernel_spmd
from concourse.masks import make_identity

F32 = mybir.dt.float32
F32R = mybir.dt.float32r
ADD = mybir.AluOpType.add
MULT = mybir.AluOpType.mult
EXP = mybir.ActivationFunctionType.Exp
SQRT = mybir.ActivationFunctionType.Sqrt
SQUARE = mybir.ActivationFunctionType.Square

EPS = 1e-8
NEG = -50.0  # additive pre-exp mask; exp(-50 + |s|max~11) ~ 1e-17


def full_cfg():
    return dict(B=2, S=2048, E=2048, D=128, G=2)


def derived(cfg):
    B, S, E, D, G = cfg["B"], cfg["S"], cfg["E"], cfg["D"], cfg["G"]
    NH = E // D            # query heads
    ET = E // 128          # 128-row tiles of E (contraction / feature tiles)
    NKT = S // 128         # key tiles
    QPC = S // 4           # query tokens per core (2 chunks)
    CH = S // 8            # chunk size
    TD = CH // 128         # diagonal key-tiles per chunk
    GS = NH // G           # heads per kv group
    assert D == 128 and CH % 128 == 0 and QPC <= 512
    return NH, ET, NKT, QPC, CH, TD, GS


def build_program(cfg):
    B, S, E, D, G = cfg["B"], cfg["S"], cfg["E"], cfg["D"], cfg["G"]
    NH, ET, NKT, QPC, CH, TD, GS = derived(cfg)
    SCALE = 1.0 / math.sqrt(D)
    KC = 512               # key-column chunk width for projections
    NKC = S // KC

    nc = bacc.Bacc()
    xT_d = nc.dram_tensor("xT", [E, S], F32, kind="ExternalInput")
    wq_d = nc.dram_tensor("Wq", [E, E], F32, kind="ExternalInput")
    wk_d = nc.dram_tensor("Wk", [E, G * D], F32, kind="ExternalInput")
    wv_d = nc.dram_tensor("Wv", [E, G * D], F32, kind="ExternalInput")
    wo_d = nc.dram_tensor("Wo", [E, E], F32, kind="ExternalInput")
    bq_d = nc.dram_tensor("bq_t", [128, ET], F32, kind="ExternalInput")
    bk_d = nc.dram_tensor("bk_t", [128, G], F32, kind="ExternalInput")
    bv_d = nc.dram_tensor("bv_t", [128, G], F32, kind="ExternalInput")
    bo_d = nc.dram_tensor("bo_t", [128, ET], F32, kind="ExternalInput")
    gq_d = nc.dram_tensor("gq_r", [1, 128], F32, kind="ExternalInput")
    gk_d = nc.dram_tensor("gk_r", [1, 128], F32, kind="ExternalInput")
    mask_d = nc.dram_tensor("mask", [TD * 128, CH], F32, kind="ExternalInput")
    bcol_d = nc.dram_tensor("bcol", [128, 2 * NKT], F32, kind="ExternalInput")
    ones_d = nc.dram_tensor("ones1", [128, 1], F32, kind="ExternalInput")
    out_d = nc.dram_tensor("outT", [E, QPC], F32, kind="ExternalOutput")

    wq_r = wq_d.rearrange("(t p) c -> p t c", p=128)   # [128, ET, E]
    wk_r = wk_d.rearrange("(t p) c -> p t c", p=128)   # [128, ET, G*D]
    wv_r = wv_d.rearrange("(t p) c -> p t c", p=128)
    wo_r = wo_d.rearrange("(t p) c -> p t c", p=128)

    def r(ap):
        return ap if ap.dtype == F32R else ap.bitcast(F32R)

    with tile.TileContext(nc) as tc, ExitStack() as top:
        consts = top.enter_context(tc.tile_pool(name="consts", bufs=1))
        persist = top.enter_context(tc.tile_pool(name="persist", bufs=1))

        ident = consts.tile([128, 128], F32)
        make_identity(nc, ident)
        ones_col = consts.tile([128, 1], F32R)
        nc.sync.dma_start(out=ones_col, in_=ones_d[:, :].bitcast(F32R))
        ones_row = consts.tile([1, 128], F32)
        nc.vector.memset(ones_row, 1.0)
        eps_t = consts.tile([1, 1], F32)
        nc.vector.memset(eps_t, EPS)
        gq_sb = consts.tile([1, 128], F32)
        nc.sync.dma_start(out=gq_sb, in_=gq_d[:, :])
        gk_sb = consts.tile([1, 128], F32)
        nc.sync.dma_start(out=gk_sb, in_=gk_d[:, :])
        bq_sb = consts.tile([128, ET], F32)
        nc.sync.dma_start(out=bq_sb, in_=bq_d[:, :])
        bk_sb = consts.tile([128, G], F32)
        nc.sync.dma_start(out=bk_sb, in_=bk_d[:, :])
        bv_sb = consts.tile([128, G], F32)
        nc.sync.dma_start(out=bv_sb, in_=bv_d[:, :])
        bo_sb = consts.tile([128, ET], F32)
        nc.sync.dma_start(out=bo_sb, in_=bo_d[:, :])
        bcol_sb = consts.tile([128, 2 * NKT], F32)
        nc.sync.dma_start(out=bcol_sb, in_=bcol_d[:, :])
        mask_sb = []
        for t in range(TD):
            m = consts.tile([128, CH], F32R, tag=f"mask{t}", name=f"mask{t}")
            nc.sync.dma_start(out=m, in_=mask_d[t * 128:(t + 1) * 128, :].bitcast(F32R))
            mask_sb.append(m)

        ktn = [persist.tile([128, S], F32R, tag=f"ktn{g}", name=f"ktn{g}") for g in range(G)]
        vtok = [persist.tile([128, NKT, 128], F32R, tag=f"vtok{g}", name=f"vtok{g}") for g in range(G)]
        qtn = persist.tile([128, NH, QPC], F32R, tag="qtn")

        # ---------------- phase 1+2: projections ------------------------
        with ExitStack() as p12:
            wkvp = p12.enter_context(tc.tile_pool(name="wkv", bufs=1))
            xsp = p12.enter_context(tc.tile_pool(name="xs", bufs=6))
            xqp = p12.enter_context(tc.tile_pool(name="xqp", bufs=1))
            tmp = p12.enter_context(tc.tile_pool(name="tmp12", bufs=3))
            wqp = p12.enter_context(tc.tile_pool(name="wqs", bufs=2))
            pkv = p12.enter_context(tc.tile_pool(name="pkv", bufs=4, space="PSUM"))
            pssq = p12.enter_context(tc.tile_pool(name="pssq", bufs=2, space="PSUM"))
            pbc = p12.enter_context(tc.tile_pool(name="pbc", bufs=2, space="PSUM"))

            wk_sb = wkvp.tile([128, ET, G * D], F32R, tag="wk")
            nc.sync.dma_start(out=wk_sb, in_=wk_r.bitcast(F32R))
            wv_sb = wkvp.tile([128, ET, G * D], F32R, tag="wv")
            nc.sync.dma_start(out=wv_sb, in_=wv_r.bitcast(F32R))

            # one step of deferred post-processing per (kc): list of thunks
            pending = []

            def flush():
                while pending:
                    pending.pop(0)()

            for kc in range(NKC):
                xts = []
                for et in range(ET):
                    xt = xsp.tile([128, KC], F32R, tag="xt")
                    nc.sync.dma_start(
                        out=xt, in_=xT_d[et * 128:(et + 1) * 128,
                                         kc * KC:(kc + 1) * KC].bitcast(F32R))
                    xts.append(xt)
                accs = []
                for ci in range(2 * G):  # K g0, K g1, V g0, V g1
                    acc = pkv.tile([128, KC], F32, tag="pkv", name="acc")
                    accs.append(acc)
                for et in range(ET):
                    for ci in range(2 * G):
                        w_sb = wk_sb if ci < G else wv_sb
                        g = ci % G
                        nc.tensor.matmul(
                            accs[ci],
                            lhsT=r(w_sb[:, et, g * D:(g + 1) * D]),
                            rhs=r(xts[et]),
                            start=(et == 0), stop=(et == ET - 1))
                flush()

                def post_kv(kc=kc, accs=accs):
                    for ci in range(2 * G):
                        g = ci % G
                        is_k = ci < G
                        bsb = bk_sb if is_k else bv_sb
                        vb = tmp.tile([128, KC], F32, tag="vb", name="vb")
                        nc.vector.tensor_scalar(
                            out=vb, in0=accs[ci], scalar1=bsb[:, g:g + 1],
                            scalar2=None, op0=ADD)
                        if is_k:
                            sq = tmp.tile([128, KC], F32R, tag="sq", name="sq")
                            nc.scalar.activation(out=sq, in_=vb, func=SQUARE)
                            ssq = pssq.tile([1, KC], F32, tag="ssq", name="ssq")
                            nc.tensor.matmul(ssq, lhsT=r(ones_col), rhs=r(sq),
                                             start=True, stop=True)
                            rms = tmp.tile([1, KC], F32, tag="rms", name="rms")
                            nc.scalar.activation(out=rms, in_=ssq, func=SQRT,
                                                 scale=1.0 / D, bias=eps_t[:, :])
                            rinv = tmp.tile([1, KC], F32, tag="rinv", name="rinv")
                            nc.vector.reciprocal(out=rinv, in_=rms)
                            bc = pbc.tile([128, KC], F32, tag="bc", name="bc")
                            nc.tensor.matmul(bc, lhsT=gk_sb, rhs=rinv,
                                             start=True, stop=True)
                            nc.vector.tensor_tensor(
                                out=ktn[g][:, kc * KC:(kc + 1) * KC],
                                in0=vb, in1=bc, op=MULT)
                        else:
                            for s in range(KC // 128):
                                vt = pbc.tile([128, 128], F32, tag="bc",
                                              name="vt")
                                nc.tensor.transpose(
                                    vt, in_=vb[:, s * 128:(s + 1) * 128],
                                    identity=ident)
                                kt_i = (kc * KC) // 128 + s
                                nc.scalar.copy(out=vtok[g][:, kt_i, :], in_=vt)
                pending.append(post_kv)
            flush()

            # ---- phase 2: Q projection (query cols are xT[:, 0:QPC]) ----
            xq = []
            for et in range(ET):
                xt = xqp.tile([128, QPC], F32R, tag=f"xq{et}", name=f"xq{et}")
                nc.sync.dma_start(
                    out=xt, in_=xT_d[et * 128:(et + 1) * 128, 0:QPC].bitcast(F32R))
                xq.append(xt)
            for qc in range(NH):
                wq_sb = wqp.tile([128, ET, 128], F32R, tag="wq", name="wq")
                nc.sync.dma_start(
                    out=wq_sb, in_=wq_r[:, :, qc * 128:(qc + 1) * 128].bitcast(F32R))
                acc = pkv.tile([128, QPC], F32, tag="pkv", name="qacc")
                for et in range(ET):
                    nc.tensor.matmul(acc, lhsT=r(wq_sb[:, et, :]),
                                     rhs=r(xq[et]),
                                     start=(et == 0), stop=(et == ET - 1))

                def post_q(qc=qc, acc=acc):
                    vb = tmp.tile([128, QPC], F32, tag="vb", name="qb")
                    nc.vector.tensor_scalar(
                        out=vb, in0=acc, scalar1=bq_sb[:, qc:qc + 1],
                        scalar2=None, op0=ADD)
                    sq = tmp.tile([128, QPC], F32R, tag="sq", name="qsq")
                    nc.scalar.activation(out=sq, in_=vb, func=SQUARE)
                    ssq = pssq.tile([1, QPC], F32, tag="ssq", name="qssq")
                    nc.tensor.matmul(ssq, lhsT=r(ones_col), rhs=r(sq),
                                     start=True, stop=True)
                    rms = tmp.tile([1, QPC], F32, tag="rms", name="qrms")
                    nc.scalar.activation(out=rms, in_=ssq, func=SQRT,
                                         scale=1.0 / D, bias=eps_t[:, :])
                    rinv = tmp.tile([1, QPC], F32, tag="rinv", name="qrinv")
                    nc.vector.reciprocal(out=rinv, in_=rms)
                    bc = pbc.tile([128, QPC], F32, tag="bc", name="qbc")
                    nc.tensor.matmul(bc, lhsT=gq_sb, rhs=rinv,
                                     start=True, stop=True)
                    nc.vector.tensor_tensor(out=qtn[:, qc, :], in0=vb,
                                            in1=bc, op=MULT)
                pending.append(post_q)
                if qc >= 1:
                    pending.pop(0)()
            flush()

        # ---------------- phase 3: attention + phase 4: out proj --------
        with ExitStack() as p34:
            ctxp = p34.enter_context(tc.tile_pool(name="ctxp", bufs=1))
            ctxt = ctxp.tile([128, ET, QPC], F32R, tag="ctxt", name="ctxt")
            ptp = p34.enter_context(tc.tile_pool(name="pt", bufs=4))
            wop = p34.enter_context(tc.tile_pool(name="wos", bufs=3))
            osb = p34.enter_context(tc.tile_pool(name="osb", bufs=3))
            psc = p34.enter_context(tc.tile_pool(name="psc", bufs=2, space="PSUM"))
            pden = p34.enter_context(tc.tile_pool(name="pden", bufs=2, space="PSUM"))
            pcx = p34.enter_context(tc.tile_pool(name="pcx", bufs=3, space="PSUM"))
            pbc2 = p34.enter_context(tc.tile_pool(name="pbc2", bufs=1, space="PSUM"))
            pending2 = []

            def flush2():
                while pending2:
                    pending2.pop(0)()

            for h in range(NH):
                g = h // GS
                den = pden.tile([1, QPC], F32, tag="den", name="den")
                cx = pcx.tile([128, QPC], F32, tag="cx", name="cx")
                for kt in range(NKT):
                    sc = psc.tile([128, QPC], F32, tag="sc", name="sc")
                    nc.tensor.matmul(
                        sc, lhsT=r(ktn[g][:, kt * 128:(kt + 1) * 128]),
                        rhs=r(qtn[:, h, :]), start=True, stop=True)

                    def post_sc(h=h, g=g, kt=kt, sc=sc, den=den, cx=cx):
                        pt = ptp.tile([128, QPC], F32R, tag="pt", name="pt")
                        for half in range(2):
                            nc.scalar.activation(
                                out=pt[:, half * CH:(half + 1) * CH],
                                in_=sc[:, half * CH:(half + 1) * CH],
                                func=EXP, scale=SCALE,
                                bias=bcol_sb[:, half * NKT + kt:
                                             half * NKT + kt + 1])
                        if kt < TD:
                            nc.vector.tensor_tensor(
                                out=pt[:, 0:CH], in0=pt[:, 0:CH],
                                in1=mask_sb[kt], op=MULT)
                        elif kt < 2 * TD:
                            nc.vector.tensor_tensor(
                                out=pt[:, CH:2 * CH], in0=pt[:, CH:2 * CH],
                                in1=mask_sb[kt - TD], op=MULT)
                        nc.tensor.matmul(den, lhsT=r(ones_col), rhs=r(pt),
                                         start=(kt == 0), stop=(kt == NKT - 1))
                        nc.tensor.matmul(cx, lhsT=r(vtok[g][:, kt, :]),
                                         rhs=r(pt),
                                         start=(kt == 0), stop=(kt == NKT - 1))
                    pending2.append(post_sc)
                    if kt >= 1:
                        pending2.pop(0)()

                def post_head(h=h, den=den, cx=cx):
                    rd = ptp.tile([1, QPC], F32, tag="rd", name="rd")
                    nc.vector.reciprocal(out=rd, in_=den)
                    bc2 = pbc2.tile([128, QPC], F32, tag="bc2", name="bc2")
                    nc.tensor.matmul(bc2, lhsT=ones_row, rhs=rd,
                                     start=True, stop=True)
                    bc2s = ptp.tile([128, QPC], F32, tag="bc2s", name="bc2s")
                    nc.vector.tensor_copy(out=bc2s, in_=bc2)
                    nc.vector.tensor_tensor(out=ctxt[:, h, :], in0=cx,
                                            in1=bc2s, op=MULT)
                pending2.append(post_head)
            flush2()

            for c2 in range(ET):
                wo_sb = wop.tile([128, ET, 128], F32R, tag="wo", name="wo")
                nc.sync.dma_start(
                    out=wo_sb, in_=wo_r[:, :, c2 * 128:(c2 + 1) * 128].bitcast(F32R))
                acc = pcx.tile([128, QPC], F32, tag="cx", name="oacc")
                for ct in range(ET):
                    nc.tensor.matmul(acc, lhsT=r(wo_sb[:, ct, :]),
                                     rhs=r(ctxt[:, ct, :]),
                                     start=(ct == 0), stop=(ct == ET - 1))

                def post_o(c2=c2, acc=acc):
                    ot = osb.tile([128, QPC], F32, tag="ot", name="ot")
                    nc.vector.tensor_scalar(
                        out=ot, in0=acc, scalar1=bo_sb[:, c2:c2 + 1],
                        scalar2=None, op0=ADD)
                    nc.sync.dma_start(
                        out=out_d[c2 * 128:(c2 + 1) * 128, :], in_=ot)
                pending2.append(post_o)
                if c2 >= 1:
                    pending2.pop(0)()
            flush2()
    nc.compile()
    return nc


# ---------------------------------------------------------------------------
# host-side sharding
# ---------------------------------------------------------------------------

def core_perm(cfg, j):
    """Permutation of token positions for quarter j: [A | B | c1 | c2 | c3]."""
    S = cfg["S"]
    CH = S // 8
    A = np.arange(CH * j, CH * (j + 1))
    Bc = np.arange(S - CH * (j + 1), S - CH * j)
    rest = np.setdiff1d(np.arange(S), np.concatenate([A, Bc]))
    c1 = rest[rest < CH * j]                                # before A
    c3 = rest[rest >= S - CH * j]                           # after B
    c2 = rest[(rest >= CH * j) & (rest < S - CH * j)]       # middle
    perm = np.concatenate([A, Bc, c1, c2, c3])
    assert perm.shape == (S,)
    return perm


def core_biascol(cfg, j):
    """[128, 2*NKT] additive exp biases (0 keep / NEG drop) per k-tile."""
    S = cfg["S"]
    NKT = S // 128
    CH = S // 8
    TD = CH // 128
    bc = np.zeros((128, 2 * NKT), np.float32)
    for kt in range(NKT):
        lo = kt * 128
        # half A (queries = chunk j): valid keys are perm cols [0,CH) (tri,
        # handled by mask => bias 0) and c1 block [2CH, 2CH + CH*j)
        validA = (lo < CH) or (2 * CH <= lo < 2 * CH + CH * j)
        # half B: valid keys: A cols [0,CH), own tri [CH,2CH), c1+c2 block
        # [2CH, 2CH + CH*j + (S - 2CH - 2CH*j)) = [2CH, S - CH*j)
        validB = (lo < 2 * CH) or (2 * CH <= lo < S - CH * j)
        bc[:, kt] = 0.0 if validA else NEG
        bc[:, NKT + kt] = 0.0 if validB else NEG
    return bc


def tri_masks(cfg):
    S = cfg["S"]
    CH = S // 8
    TD = CH // 128
    m = np.zeros((TD * 128, CH), np.float32)
    for t in range(TD):
        kk = np.arange(128)[:, None] + t * 128
        qq = np.arange(CH)[None, :]
        m[t * 128:(t + 1) * 128, :] = (kk <= qq).astype(np.float32)
    return m


def make_in_maps(cfg, inputs):
    """Build the 8 per-core input dicts. Returns (in_maps, perms)."""
    B, S, E, D, G = cfg["B"], cfg["S"], cfg["E"], cfg["D"], cfg["G"]
    NH, ET, NKT, QPC, CH, TD, GS = derived(cfg)
    x = np.asarray(inputs["x"], np.float32)
    shared = dict(
        Wq=np.ascontiguousarray(inputs["Wq"], np.float32),
        Wk=np.ascontiguousarray(inputs["Wk"], np.float32),
        Wv=np.ascontiguousarray(inputs["Wv"], np.float32),
        Wo=np.ascontiguousarray(inputs["Wo"], np.float32),
        bq_t=np.ascontiguousarray(
            np.asarray(inputs["bq"], np.float32).reshape(ET, 128).T),
        bk_t=np.ascontiguousarray(
            np.asarray(inputs["bk"], np.float32).reshape(G, 128).T),
        bv_t=np.ascontiguousarray(
            np.asarray(inputs["bv"], np.float32).reshape(G, 128).T),
        bo_t=np.ascontiguousarray(
            np.asarray(inputs["bo"], np.float32).reshape(ET, 128).T),
        gq_r=np.ascontiguousarray(
            np.asarray(inputs["gamma_q"], np.float32).reshape(1, 128)),
        gk_r=np.ascontiguousarray(
            np.asarray(inputs["gamma_k"], np.float32).reshape(1, 128)),
        mask=tri_masks(cfg),
        ones1=np.ones((128, 1), np.float32),
    )
    in_maps, perms = [], []
    for c in range(8):
        b, j = c // 4, c % 4
        perm = core_perm(cfg, j)
        xt = np.ascontiguousarray(x[b].T[:, perm])  # [E, S] permuted cols
        m = dict(shared)
        m["xT"] = xt
        m["bcol"] = core_biascol(cfg, j)
        in_maps.append(m)
        perms.append(perm)
    return in_maps, perms


def assemble(cfg, results, perms):
    B, S, E = cfg["B"], cfg["S"], cfg["E"]
    QPC = S // 4
    out = np.empty((B, S, E), np.float32)
    for c in range(8):
        b = c // 4
        out[b, perms[c][:QPC], :] = results[c]["outT"].T
    return out


_CACHE = {}


def kernel(**inputs):
    cfg = full_cfg()
    if "nc" not in _CACHE:
        _CACHE["nc"] = build_program(cfg)
    nc = _CACHE["nc"]
    in_maps, perms = make_in_maps(cfg, inputs)
    res = run_bass_kernel_spmd(nc, in_maps, list(range(8)))
    return assemble(cfg, res.results, perms)



# revision 6
# speedup vs baseline: 1.2777x; 1.2777x over previous
"""GQA attention block (RMSNorm-QK, causal, GQA) on 8 trn2 NeuronCores.

Strategy: sequence sharding, zero collectives. Core c handles batch c//4 and
two causally-balanced query chunks (j and 7-j of 8) of 256 rows each. The host
permutes the key/token axis per core so every core sees its own query tokens
at fixed columns [0:512]; causality becomes per-core *data* (exp-bias columns
+ two constant triangle masks), so one uniform SPMD program serves all cores.

All activations live feature-major ("T layout", [feature, token]) so every
matmul consumes natural layouts with zero on-device transposes except V
(tiny). Scores are computed transposed ([k, q]); softmax needs no max
subtraction because RMS-normalized q,k bound |scores/sqrt(D)| <= sqrt(D).
Partition-dim reductions (RMS sum-of-squares, softmax denominators) are
rank-1 matmuls on the PE; per-token broadcasts are rank-1 matmuls as well.
"""

import math
import numpy as np
from contextlib import ExitStack

import concourse.bass as bass
import concourse.mybir as mybir
import concourse.tile as tile
from concourse import bacc
from concourse.bass_utils import run_bass_kernel_spmd
from concourse.masks import make_identity

F32 = mybir.dt.float32
F32R = mybir.dt.float32r
ADD = mybir.AluOpType.add
MULT = mybir.AluOpType.mult
EXP = mybir.ActivationFunctionType.Exp
SQRT = mybir.ActivationFunctionType.Sqrt
SQUARE = mybir.ActivationFunctionType.Square

EPS = 1e-8
NEG = -50.0  # additive pre-exp mask; exp(-50 + |s|max~11) ~ 1e-17


def full_cfg():
    return dict(B=2, S=2048, E=2048, D=128, G=2)


def derived(cfg):
    B, S, E, D, G = cfg["B"], cfg["S"], cfg["E"], cfg["D"], cfg["G"]
    NH = E // D            # query heads
    ET = E // 128          # 128-row tiles of E (contraction / feature tiles)
    NKT = S // 128         # key tiles
    QPC = S // 4           # query tokens per core (2 chunks)
    CH = S // 8            # chunk size
    TD = CH // 128         # diagonal key-tiles per chunk
    GS = NH // G           # heads per kv group
    assert D == 128 and CH % 128 == 0 and QPC <= 512
    return NH, ET, NKT, QPC, CH, TD, GS


def build_program(cfg):
    B, S, E, D, G = cfg["B"], cfg["S"], cfg["E"], cfg["D"], cfg["G"]
    NH, ET, NKT, QPC, CH, TD, GS = derived(cfg)
    SCALE = 1.0 / math.sqrt(D)
    KC = 512               # key-column chunk width for projections
    NKC = S // KC

    nc = bacc.Bacc()
    xT_d = nc.dram_tensor("xT", [E, S], F32, kind="ExternalInput")
    wq_d = nc.dram_tensor("Wq", [E, E], F32, kind="ExternalInput")
    wk_d = nc.dram_tensor("Wk", [E, G * D], F32, kind="ExternalInput")
    wv_d = nc.dram_tensor("Wv", [E, G * D], F32, kind="ExternalInput")
    wo_d = nc.dram_tensor("Wo", [E, E], F32, kind="ExternalInput")
    bq_d = nc.dram_tensor("bq_t", [128, ET], F32, kind="ExternalInput")
    bk_d = nc.dram_tensor("bk_t", [128, G], F32, kind="ExternalInput")
    bv_d = nc.dram_tensor("bv_t", [128, G], F32, kind="ExternalInput")
    bo_d = nc.dram_tensor("bo_t", [128, ET], F32, kind="ExternalInput")
    gq_d = nc.dram_tensor("gq_r", [1, 128], F32, kind="ExternalInput")
    gk_d = nc.dram_tensor("gk_r", [1, 128], F32, kind="ExternalInput")
    mask_d = nc.dram_tensor("mask", [TD * 128, CH], F32, kind="ExternalInput")
    bcol_d = nc.dram_tensor("bcol", [128, 2 * NKT], F32, kind="ExternalInput")
    ones_d = nc.dram_tensor("ones1", [128, 1], F32, kind="ExternalInput")
    out_d = nc.dram_tensor("outT", [E, QPC], F32, kind="ExternalOutput")

    wq_r = wq_d.rearrange("(t p) c -> p t c", p=128)   # [128, ET, E]
    wk_r = wk_d.rearrange("(t p) c -> p t c", p=128)   # [128, ET, G*D]
    wv_r = wv_d.rearrange("(t p) c -> p t c", p=128)
    wo_r = wo_d.rearrange("(t p) c -> p t c", p=128)

    def r(ap):
        return ap if ap.dtype == F32R else ap.bitcast(F32R)

    with tile.TileContext(nc) as tc, ExitStack() as top:
        consts = top.enter_context(tc.tile_pool(name="consts", bufs=1))
        persist = top.enter_context(tc.tile_pool(name="persist", bufs=1))

        ident = consts.tile([128, 128], F32)
        make_identity(nc, ident)
        ones_col = consts.tile([128, 1], F32R)
        nc.sync.dma_start(out=ones_col, in_=ones_d[:, :].bitcast(F32R))
        ones_row = consts.tile([1, 128], F32)
        nc.vector.memset(ones_row, 1.0)
        eps_t = consts.tile([1, 1], F32)
        nc.vector.memset(eps_t, EPS)
        gq_sb = consts.tile([1, 128], F32)
        nc.sync.dma_start(out=gq_sb, in_=gq_d[:, :])
        gk_sb = consts.tile([1, 128], F32)
        nc.sync.dma_start(out=gk_sb, in_=gk_d[:, :])
        bq_sb = consts.tile([128, ET], F32)
        nc.sync.dma_start(out=bq_sb, in_=bq_d[:, :])
        bk_sb = consts.tile([128, G], F32)
        nc.sync.dma_start(out=bk_sb, in_=bk_d[:, :])
        bv_sb = consts.tile([128, G], F32)
        nc.sync.dma_start(out=bv_sb, in_=bv_d[:, :])
        bo_sb = consts.tile([128, ET], F32)
        nc.sync.dma_start(out=bo_sb, in_=bo_d[:, :])
        bcol_sb = consts.tile([128, 2 * NKT], F32)
        nc.sync.dma_start(out=bcol_sb, in_=bcol_d[:, :])
        mask_sb = []
        for t in range(TD):
            m = consts.tile([128, CH], F32R, tag=f"mask{t}", name=f"mask{t}")
            nc.sync.dma_start(out=m, in_=mask_d[t * 128:(t + 1) * 128, :].bitcast(F32R))
            mask_sb.append(m)

        ktn = [persist.tile([128, S], F32R, tag=f"ktn{g}", name=f"ktn{g}") for g in range(G)]
        vtok = [persist.tile([128, NKT, 128], F32R, tag=f"vtok{g}", name=f"vtok{g}") for g in range(G)]
        qtn = persist.tile([128, NH, QPC], F32R, tag="qtn")

        # ---------------- phase 1+2: projections ------------------------
        with ExitStack() as p12:
            wkvp = p12.enter_context(tc.tile_pool(name="wkv", bufs=1))
            xsp = p12.enter_context(tc.tile_pool(name="xs", bufs=6))
            xqp = p12.enter_context(tc.tile_pool(name="xqp", bufs=1))
            tmp = p12.enter_context(tc.tile_pool(name="tmp12", bufs=3))
            wqp = p12.enter_context(tc.tile_pool(name="wqs", bufs=2))
            pkv = p12.enter_context(tc.tile_pool(name="pkv", bufs=4, space="PSUM"))
            pssq = p12.enter_context(tc.tile_pool(name="pssq", bufs=2, space="PSUM"))
            pbc = p12.enter_context(tc.tile_pool(name="pbc", bufs=2, space="PSUM"))

            wk_sb = wkvp.tile([128, ET, G * D], F32R, tag="wk")
            nc.sync.dma_start(out=wk_sb, in_=wk_r.bitcast(F32R))
            wv_sb = wkvp.tile([128, ET, G * D], F32R, tag="wv")
            nc.sync.dma_start(out=wv_sb, in_=wv_r.bitcast(F32R))

            # one step of deferred post-processing per (kc): list of thunks
            pending = []

            def flush():
                while pending:
                    pending.pop(0)()

            for kc in range(NKC):
                xts = []
                for et in range(ET):
                    xt = xsp.tile([128, KC], F32R, tag="xt")
                    nc.sync.dma_start(
                        out=xt, in_=xT_d[et * 128:(et + 1) * 128,
                                         kc * KC:(kc + 1) * KC].bitcast(F32R))
                    xts.append(xt)
                accs = []
                for ci in range(2 * G):  # K g0, K g1, V g0, V g1
                    acc = pkv.tile([128, KC], F32, tag="pkv", name="acc")
                    accs.append(acc)
                for et in range(ET):
                    for ci in range(2 * G):
                        w_sb = wk_sb if ci < G else wv_sb
                        g = ci % G
                        nc.tensor.matmul(
                            accs[ci],
                            lhsT=r(w_sb[:, et, g * D:(g + 1) * D]),
                            rhs=r(xts[et]),
                            start=(et == 0), stop=(et == ET - 1))
                flush()

                def post_kv(kc=kc, accs=accs):
                    for ci in range(2 * G):
                        g = ci % G
                        is_k = ci < G
                        bsb = bk_sb if is_k else bv_sb
                        vb = tmp.tile([128, KC], F32, tag="vb", name="vb")
                        nc.vector.tensor_scalar(
                            out=vb, in0=accs[ci], scalar1=bsb[:, g:g + 1],
                            scalar2=None, op0=ADD)
                        if is_k:
                            sq = tmp.tile([128, KC], F32R, tag="sq", name="sq")
                            nc.scalar.activation(out=sq, in_=vb, func=SQUARE)
                            ssq = pssq.tile([1, KC], F32, tag="ssq", name="ssq")
                            nc.tensor.matmul(ssq, lhsT=r(ones_col), rhs=r(sq),
                                             start=True, stop=True)
                            rms = tmp.tile([1, KC], F32, tag="rms", name="rms")
                            nc.scalar.activation(out=rms, in_=ssq, func=SQRT,
                                                 scale=1.0 / D, bias=eps_t[:, :])
                            rinv = tmp.tile([1, KC], F32, tag="rinv", name="rinv")
                            nc.vector.reciprocal_approx_fast(out=rinv, in_=rms)
                            bc = pbc.tile([128, KC], F32, tag="bc", name="bc")
                            nc.tensor.matmul(bc, lhsT=gk_sb, rhs=rinv,
                                             start=True, stop=True)
                            nc.vector.tensor_tensor(
                                out=ktn[g][:, kc * KC:(kc + 1) * KC],
                                in0=vb, in1=bc, op=MULT)
                        else:
                            for s in range(KC // 128):
                                vt = pbc.tile([128, 128], F32, tag="bc",
                                              name="vt")
                                nc.tensor.transpose(
                                    vt, in_=vb[:, s * 128:(s + 1) * 128],
                                    identity=ident)
                                kt_i = (kc * KC) // 128 + s
                                nc.scalar.copy(out=vtok[g][:, kt_i, :], in_=vt)
                pending.append(post_kv)
            flush()

            # ---- phase 2: Q projection (query cols are xT[:, 0:QPC]) ----
            xq = []
            for et in range(ET):
                xt = xqp.tile([128, QPC], F32R, tag=f"xq{et}", name=f"xq{et}")
                nc.sync.dma_start(
                    out=xt, in_=xT_d[et * 128:(et + 1) * 128, 0:QPC].bitcast(F32R))
                xq.append(xt)
            for qc in range(NH):
                wq_sb = wqp.tile([128, ET, 128], F32R, tag="wq", name="wq")
                nc.sync.dma_start(
                    out=wq_sb, in_=wq_r[:, :, qc * 128:(qc + 1) * 128].bitcast(F32R))
                acc = pkv.tile([128, QPC], F32, tag="pkv", name="qacc")
                for et in range(ET):
                    nc.tensor.matmul(acc, lhsT=r(wq_sb[:, et, :]),
                                     rhs=r(xq[et]),
                                     start=(et == 0), stop=(et == ET - 1))

                def post_q(qc=qc, acc=acc):
                    vb = tmp.tile([128, QPC], F32, tag="vb", name="qb")
                    nc.vector.tensor_scalar(
                        out=vb, in0=acc, scalar1=bq_sb[:, qc:qc + 1],
                        scalar2=None, op0=ADD)
                    sq = tmp.tile([128, QPC], F32R, tag="sq", name="qsq")
                    nc.scalar.activation(out=sq, in_=vb, func=SQUARE)
                    ssq = pssq.tile([1, QPC], F32, tag="ssq", name="qssq")
                    nc.tensor.matmul(ssq, lhsT=r(ones_col), rhs=r(sq),
                                     start=True, stop=True)
                    rms = tmp.tile([1, QPC], F32, tag="rms", name="qrms")
                    nc.scalar.activation(out=rms, in_=ssq, func=SQRT,
                                         scale=1.0 / D, bias=eps_t[:, :])
                    rinv = tmp.tile([1, QPC], F32, tag="rinv", name="qrinv")
                    nc.vector.reciprocal_approx_fast(out=rinv, in_=rms)
                    bc = pbc.tile([128, QPC], F32, tag="bc", name="qbc")
                    nc.tensor.matmul(bc, lhsT=gq_sb, rhs=rinv,
                                     start=True, stop=True)
                    nc.vector.tensor_tensor(out=qtn[:, qc, :], in0=vb,
                                            in1=bc, op=MULT)
                pending.append(post_q)
                if qc >= 1:
                    pending.pop(0)()
            flush()

        # ---------------- phase 3: attention + phase 4: out proj --------
        with ExitStack() as p34:
            ctxp = p34.enter_context(tc.tile_pool(name="ctxp", bufs=1))
            ctxt = ctxp.tile([128, ET, QPC], F32R, tag="ctxt", name="ctxt")
            ptp = p34.enter_context(tc.tile_pool(name="pt", bufs=4))
            wop = p34.enter_context(tc.tile_pool(name="wos", bufs=3))
            osb = p34.enter_context(tc.tile_pool(name="osb", bufs=3))
            psc = p34.enter_context(tc.tile_pool(name="psc", bufs=3, space="PSUM"))
            pden = p34.enter_context(tc.tile_pool(name="pden", bufs=2, space="PSUM"))
            pcx = p34.enter_context(tc.tile_pool(name="pcx", bufs=2, space="PSUM"))
            pbc2 = p34.enter_context(tc.tile_pool(name="pbc2", bufs=1, space="PSUM"))
            pending2 = []

            def flush2():
                while pending2:
                    pending2.pop(0)()

            for h in range(NH):
                g = h // GS
                den = pden.tile([1, QPC], F32, tag="den", name="den")
                cx = pcx.tile([128, QPC], F32, tag="cx", name="cx")
                for kt in range(NKT):
                    sc = psc.tile([128, QPC], F32, tag="sc", name="sc")
                    nc.tensor.matmul(
                        sc, lhsT=r(ktn[g][:, kt * 128:(kt + 1) * 128]),
                        rhs=r(qtn[:, h, :]), start=True, stop=True)

                    def post_sc(h=h, g=g, kt=kt, sc=sc, den=den, cx=cx):
                        pt = ptp.tile([128, QPC], F32R, tag="pt", name="pt")
                        for half in range(2):
                            nc.scalar.activation(
                                out=pt[:, half * CH:(half + 1) * CH],
                                in_=sc[:, half * CH:(half + 1) * CH],
                                func=EXP, scale=SCALE,
                                bias=bcol_sb[:, half * NKT + kt:
                                             half * NKT + kt + 1])
                        if kt < TD:
                            nc.vector.tensor_tensor(
                                out=pt[:, 0:CH], in0=pt[:, 0:CH],
                                in1=mask_sb[kt], op=MULT)
                        elif kt < 2 * TD:
                            nc.vector.tensor_tensor(
                                out=pt[:, CH:2 * CH], in0=pt[:, CH:2 * CH],
                                in1=mask_sb[kt - TD], op=MULT)
                        nc.tensor.matmul(den, lhsT=r(ones_col), rhs=r(pt),
                                         start=(kt == 0), stop=(kt == NKT - 1))
                        nc.tensor.matmul(cx, lhsT=r(vtok[g][:, kt, :]),
                                         rhs=r(pt),
                                         start=(kt == 0), stop=(kt == NKT - 1))
                    pending2.append(post_sc)
                    if kt >= 2:
                        pending2.pop(0)()

                def post_head(h=h, den=den, cx=cx):
                    rd = ptp.tile([1, QPC], F32, tag="rd", name="rd")
                    nc.vector.reciprocal_approx_fast(out=rd, in_=den)
                    bc2 = pbc2.tile([128, QPC], F32, tag="bc2", name="bc2")
                    nc.tensor.matmul(bc2, lhsT=ones_row, rhs=rd,
                                     start=True, stop=True)
                    bc2s = ptp.tile([128, QPC], F32, tag="bc2s", name="bc2s")
                    nc.vector.tensor_copy(out=bc2s, in_=bc2)
                    nc.vector.tensor_tensor(out=ctxt[:, h, :], in0=cx,
                                            in1=bc2s, op=MULT)
                pending2.append(post_head)
            flush2()

            for c2 in range(ET):
                wo_sb = wop.tile([128, ET, 128], F32R, tag="wo", name="wo")
                nc.sync.dma_start(
                    out=wo_sb, in_=wo_r[:, :, c2 * 128:(c2 + 1) * 128].bitcast(F32R))
                acc = pcx.tile([128, QPC], F32, tag="cx", name="oacc")
                for ct in range(ET):
                    nc.tensor.matmul(acc, lhsT=r(wo_sb[:, ct, :]),
                                     rhs=r(ctxt[:, ct, :]),
                                     start=(ct == 0), stop=(ct == ET - 1))

                def post_o(c2=c2, acc=acc):
                    ot = osb.tile([128, QPC], F32, tag="ot", name="ot")
                    nc.vector.tensor_scalar(
                        out=ot, in0=acc, scalar1=bo_sb[:, c2:c2 + 1],
                        scalar2=None, op0=ADD)
                    nc.sync.dma_start(
                        out=out_d[c2 * 128:(c2 + 1) * 128, :], in_=ot)
                pending2.append(post_o)
                if c2 >= 1:
                    pending2.pop(0)()
            flush2()
    nc.compile()
    return nc


# ---------------------------------------------------------------------------
# host-side sharding
# ---------------------------------------------------------------------------

def core_perm(cfg, j):
    """Permutation of token positions for quarter j: [A | B | c1 | c2 | c3]."""
    S = cfg["S"]
    CH = S // 8
    A = np.arange(CH * j, CH * (j + 1))
    Bc = np.arange(S - CH * (j + 1), S - CH * j)
    rest = np.setdiff1d(np.arange(S), np.concatenate([A, Bc]))
    c1 = rest[rest < CH * j]                                # before A
    c3 = rest[rest >= S - CH * j]                           # after B
    c2 = rest[(rest >= CH * j) & (rest < S - CH * j)]       # middle
    perm = np.concatenate([A, Bc, c1, c2, c3])
    assert perm.shape == (S,)
    return perm


def core_biascol(cfg, j):
    """[128, 2*NKT] additive exp biases (0 keep / NEG drop) per k-tile."""
    S = cfg["S"]
    NKT = S // 128
    CH = S // 8
    TD = CH // 128
    bc = np.zeros((128, 2 * NKT), np.float32)
    for kt in range(NKT):
        lo = kt * 128
        # half A (queries = chunk j): valid keys are perm cols [0,CH) (tri,
        # handled by mask => bias 0) and c1 block [2CH, 2CH + CH*j)
        validA = (lo < CH) or (2 * CH <= lo < 2 * CH + CH * j)
        # half B: valid keys: A cols [0,CH), own tri [CH,2CH), c1+c2 block
        # [2CH, 2CH + CH*j + (S - 2CH - 2CH*j)) = [2CH, S - CH*j)
        validB = (lo < 2 * CH) or (2 * CH <= lo < S - CH * j)
        bc[:, kt] = 0.0 if validA else NEG
        bc[:, NKT + kt] = 0.0 if validB else NEG
    return bc


def tri_masks(cfg):
    S = cfg["S"]
    CH = S // 8
    TD = CH // 128
    m = np.zeros((TD * 128, CH), np.float32)
    for t in range(TD):
        kk = np.arange(128)[:, None] + t * 128
        qq = np.arange(CH)[None, :]
        m[t * 128:(t + 1) * 128, :] = (kk <= qq).astype(np.float32)
    return m


def make_in_maps(cfg, inputs):
    """Build the 8 per-core input dicts. Returns (in_maps, perms)."""
    B, S, E, D, G = cfg["B"], cfg["S"], cfg["E"], cfg["D"], cfg["G"]
    NH, ET, NKT, QPC, CH, TD, GS = derived(cfg)
    x = np.asarray(inputs["x"], np.float32)
    shared = dict(
        Wq=np.ascontiguousarray(inputs["Wq"], np.float32),
        Wk=np.ascontiguousarray(inputs["Wk"], np.float32),
        Wv=np.ascontiguousarray(inputs["Wv"], np.float32),
        Wo=np.ascontiguousarray(inputs["Wo"], np.float32),
        bq_t=np.ascontiguousarray(
            np.asarray(inputs["bq"], np.float32).reshape(ET, 128).T),
        bk_t=np.ascontiguousarray(
            np.asarray(inputs["bk"], np.float32).reshape(G, 128).T),
        bv_t=np.ascontiguousarray(
            np.asarray(inputs["bv"], np.float32).reshape(G, 128).T),
        bo_t=np.ascontiguousarray(
            np.asarray(inputs["bo"], np.float32).reshape(ET, 128).T),
        gq_r=np.ascontiguousarray(
            np.asarray(inputs["gamma_q"], np.float32).reshape(1, 128)),
        gk_r=np.ascontiguousarray(
            np.asarray(inputs["gamma_k"], np.float32).reshape(1, 128)),
        mask=tri_masks(cfg),
        ones1=np.ones((128, 1), np.float32),
    )
    in_maps, perms = [], []
    for c in range(8):
        b, j = c // 4, c % 4
        perm = core_perm(cfg, j)
        xt = np.ascontiguousarray(x[b].T[:, perm])  # [E, S] permuted cols
        m = dict(shared)
        m["xT"] = xt
        m["bcol"] = core_biascol(cfg, j)
        in_maps.append(m)
        perms.append(perm)
    return in_maps, perms


def assemble(cfg, results, perms):
    B, S, E = cfg["B"], cfg["S"], cfg["E"]
    QPC = S // 4
    out = np.empty((B, S, E), np.float32)
    for c in range(8):
        b = c // 4
        out[b, perms[c][:QPC], :] = results[c]["outT"].T
    return out


_CACHE = {}


def kernel(**inputs):
    cfg = full_cfg()
    if "nc" not in _CACHE:
        _CACHE["nc"] = build_program(cfg)
    nc = _CACHE["nc"]
    in_maps, perms = make_in_maps(cfg, inputs)
    res = run_bass_kernel_spmd(nc, in_maps, list(range(8)))
    return assemble(cfg, res.results, perms)



# revision 8
# speedup vs baseline: 1.8144x; 1.4201x over previous
"""GQA attention block (RMSNorm-QK, causal, GQA) on 8 trn2 NeuronCores — v2.

Sharding: batch over groups of 4 cores; stride-4 query interleave within a
batch. Core c handles batch c//4 and query tokens {j, j+4, ..., j+2044}
(j = c%4), so the causal structure is IDENTICAL on every core: for key tile
kt (128 keys), query columns < 32*kt are fully masked (skipped entirely),
columns [32kt, 32kt+32) are diagonal (one shared [128,32] 0/1 mask), and
the rest are fully valid. Scores / exp / denominator / AV all run on the
causally-valid suffix [32kt, 512) only — ~47% less attention work than the
full rectangle, with zero collectives and one uniform SPMD program.

All matmuls are bf16 (1 cycle/row at any free size on the PE; error budget
2e-2 >> bf16's ~1e-3). Activations are feature-major ("T layout"); V is
projected token-major directly (x-tiles stationary), so the kernel needs no
transposes at all. Partition-dim reductions (RMS sum-of-squares, softmax
denominators) and per-token broadcasts are rank-1 matmuls; reciprocals use
the fast custom-DVE op (~18 bits, ~5x faster than nc.vector.reciprocal).
Score tiles for key-tile pairs (p, 16-p) pack into a single PSUM bank so
exp runs as one activation per bank. Softmax needs no max subtraction:
RMS-normalized q,k bound |scores|/sqrt(D) <= sqrt(D).
"""

import math
import numpy as np
from contextlib import ExitStack

import ml_dtypes
import concourse.bass as bass
import concourse.mybir as mybir
import concourse.tile as tile
from concourse import bacc
from concourse.bass_utils import run_bass_kernel_spmd

F32 = mybir.dt.float32
F32R = mybir.dt.float32r
BF16 = mybir.dt.bfloat16
ADD = mybir.AluOpType.add
MULT = mybir.AluOpType.mult
EXP = mybir.ActivationFunctionType.Exp
SQRT = mybir.ActivationFunctionType.Sqrt
SQUARE = mybir.ActivationFunctionType.Square

BF = ml_dtypes.bfloat16
EPS = 1e-8


def full_cfg():
    return dict(B=2, S=2048, E=2048, D=128, G=2)


def derived(cfg):
    B, S, E, D, G = cfg["B"], cfg["S"], cfg["E"], cfg["D"], cfg["G"]
    NH = E // D            # 16 query heads == E blocks of 128
    ET = E // 128          # 16 contraction tiles of E
    NKT = S // 128         # 16 key tiles
    QPC = S // 4           # 512 queries per core (stride-4 stripe)
    GS = NH // G           # 8 heads per kv group
    assert D == 128 and QPC == 512
    return NH, ET, NKT, QPC, GS


# key-tile pairs that pack into one PSUM bank: widths (512-32p) + 32p = 512
SC_TILES = [(0,), (1, 15), (2, 14), (3, 13), (4, 12), (5, 11), (6, 10),
            (7, 9), (8,)]


def build_program(cfg):
    B, S, E, D, G = cfg["B"], cfg["S"], cfg["E"], cfg["D"], cfg["G"]
    NH, ET, NKT, QPC, GS = derived(cfg)
    SCALE = 1.0 / math.sqrt(D)
    KC = 512
    NKC = S // KC

    nc = bacc.Bacc()
    xT_d = nc.dram_tensor("xT", [E, S], BF16, kind="ExternalInput")
    xq_d = nc.dram_tensor("xq", [E, QPC], BF16, kind="ExternalInput")
    wq_d = nc.dram_tensor("WqP", [128, NH, ET * 128], BF16, kind="ExternalInput")
    wo_d = nc.dram_tensor("WoP", [128, ET, ET * 128], BF16, kind="ExternalInput")
    wk_d = nc.dram_tensor("WkP", [128, ET * G * 128], BF16, kind="ExternalInput")
    wv_d = nc.dram_tensor("WvP", [128, ET * G * 128], BF16, kind="ExternalInput")
    bq_d = nc.dram_tensor("bq_t", [128, NH], F32, kind="ExternalInput")
    bk_d = nc.dram_tensor("bk_t", [128, G], F32, kind="ExternalInput")
    bv_d = nc.dram_tensor("bv_r", [1, G * 128], BF16, kind="ExternalInput")
    bo_d = nc.dram_tensor("bo_t", [128, ET], F32, kind="ExternalInput")
    gq_d = nc.dram_tensor("gq_r", [1, 128], F32, kind="ExternalInput")
    gk_d = nc.dram_tensor("gk_r", [1, 128], F32, kind="ExternalInput")
    dm_d = nc.dram_tensor("dmask", [128, 32], BF16, kind="ExternalInput")
    onc_d = nc.dram_tensor("ones_c", [128, 1], F32, kind="ExternalInput")
    onr_d = nc.dram_tensor("ones_r", [1, 128], F32, kind="ExternalInput")
    out_d = nc.dram_tensor("outT", [E, QPC], F32, kind="ExternalOutput")

    def r(ap):
        return ap if ap.dtype == F32R else ap.bitcast(F32R)

    with tile.TileContext(nc) as tc, ExitStack() as top:
        consts = top.enter_context(tc.tile_pool(name="consts", bufs=1))
        persist = top.enter_context(tc.tile_pool(name="persist", bufs=1))

        ones_col_bf = consts.tile([128, 1], BF16)
        nc.vector.memset(ones_col_bf, 1.0)
        ones_row_bf = consts.tile([1, 128], BF16)
        nc.vector.memset(ones_row_bf, 1.0)
        ones_col_r = consts.tile([128, 1], F32R)
        nc.sync.dma_start(out=ones_col_r, in_=onc_d[:, :].bitcast(F32R))
        ones_row_r = consts.tile([1, 128], F32R)
        nc.sync.dma_start(out=ones_row_r, in_=onr_d[:, :].bitcast(F32R))
        eps_t = consts.tile([1, 1], F32)
        nc.vector.memset(eps_t, EPS)
        gq_sb = consts.tile([1, 128], F32R)
        nc.sync.dma_start(out=gq_sb, in_=gq_d[:, :].bitcast(F32R))
        gk_sb = consts.tile([1, 128], F32R)
        nc.sync.dma_start(out=gk_sb, in_=gk_d[:, :].bitcast(F32R))
        bq_sb = consts.tile([128, NH], F32)
        nc.sync.dma_start(out=bq_sb, in_=bq_d[:, :])
        bk_sb = consts.tile([128, G], F32)
        nc.sync.dma_start(out=bk_sb, in_=bk_d[:, :])
        bv_sb = consts.tile([1, G * 128], BF16)
        nc.sync.dma_start(out=bv_sb, in_=bv_d[:, :])
        bo_sb = consts.tile([128, ET], F32)
        nc.sync.dma_start(out=bo_sb, in_=bo_d[:, :])
        dmask = consts.tile([128, 32], BF16)
        nc.sync.dma_start(out=dmask, in_=dm_d[:, :])

        ktn = [persist.tile([128, S], BF16, tag=f"ktn{g}", name=f"ktn{g}")
               for g in range(G)]
        vtok = persist.tile([128, NKT, G * 128], BF16, tag="vtok")
        qtn = persist.tile([128, NH, QPC], BF16, tag="qtn")
        ctxt = persist.tile([128, ET, QPC], BF16, tag="ctxt")

        # ------------- phase 1: K/V projection over all tokens ------------
        with ExitStack() as p1:
            wkvp = p1.enter_context(tc.tile_pool(name="wkv", bufs=1))
            xsp = p1.enter_context(tc.tile_pool(name="xs", bufs=36))
            tmp = p1.enter_context(tc.tile_pool(name="tmp1", bufs=3))
            pk = p1.enter_context(tc.tile_pool(name="pk", bufs=1, space="PSUM"))
            pv = p1.enter_context(tc.tile_pool(name="pv", bufs=1, space="PSUM"))
            pssq = p1.enter_context(tc.tile_pool(name="pssq", bufs=2, space="PSUM"))
            pbc = p1.enter_context(tc.tile_pool(name="pbc", bufs=2, space="PSUM"))

            wk_sb = wkvp.tile([128, ET * G * 128], BF16, tag="wk")
            nc.sync.dma_start(out=wk_sb, in_=wk_d[:, :])
            wv_sb = wkvp.tile([128, ET * G * 128], BF16, tag="wv")
            nc.sync.dma_start(out=wv_sb, in_=wv_d[:, :])

            q_top, q_mid, q_bot = [], [], []

            def pop(q):
                if q:
                    q.pop(0)()

            for kc in range(NKC):
                xts = []
                for et in range(ET):
                    xt = xsp.tile([128, KC], BF16, tag="xt", name="xt")
                    nc.sync.dma_start(
                        out=xt, in_=xT_d[et * 128:(et + 1) * 128,
                                         kc * KC:(kc + 1) * KC])
                    xts.append(xt)
                pop(q_top)
                acck = pk.tile([128, G, KC], F32, tag="acck", name="acck")
                for g in range(G):
                    for et in range(ET):
                        nc.tensor.matmul(
                            acck[:, g, :],
                            lhsT=wk_sb[:, et * 256 + g * 128:
                                       et * 256 + (g + 1) * 128],
                            rhs=xts[et], start=(et == 0), stop=(et == ET - 1))
                pop(q_mid)
                accv = pv.tile([128, 4, G * 128], F32, tag="accv", name="accv")
                for s in range(4):
                    for et in range(ET):
                        nc.tensor.matmul(
                            accv[:, s, :],
                            lhsT=xts[et][:, s * 128:(s + 1) * 128],
                            rhs=wv_sb[:, et * 256:(et + 1) * 256],
                            start=(et == 0), stop=False)
                    nc.tensor.matmul(accv[:, s, :], lhsT=ones_row_bf,
                                     rhs=bv_sb, start=False, stop=True)
                pop(q_bot)

                def top_f(kc=kc, acck=acck, accv=accv):
                    outs = []
                    for g in range(G):
                        sq = tmp.tile([128, KC], F32R, tag="sq", name="sq",
                                      bufs=3)
                        nc.scalar.activation(out=sq, in_=acck[:, g, :],
                                             func=SQUARE,
                                             bias=bk_sb[:, g:g + 1])
                        vb = tmp.tile([128, KC], F32, tag="vb", name="vb",
                                      bufs=5)
                        nc.vector.tensor_scalar(
                            out=vb, in0=acck[:, g, :],
                            scalar1=bk_sb[:, g:g + 1], scalar2=None, op0=ADD)
                        outs.append((sq, vb))
                    for s in range(4):
                        nc.scalar.copy(out=vtok[:, kc * 4 + s, :],
                                       in_=accv[:, s, :])
                    q_mid.append(lambda kc=kc, outs=outs: mid_f(kc, outs))

                def mid_f(kc, outs):
                    outs2 = []
                    for g in range(G):
                        sq, vb = outs[g]
                        ssq = pssq.tile([1, KC], F32, tag="ssq", name="ssq")
                        nc.tensor.matmul(ssq, lhsT=ones_col_r, rhs=sq,
                                         start=True, stop=True)
                        rms = tmp.tile([1, KC], F32, tag="rms", name="rms",
                                       bufs=3)
                        nc.scalar.activation(out=rms, in_=ssq, func=SQRT,
                                             scale=1.0 / D, bias=eps_t[:, :])
                        rinv = tmp.tile([1, KC], F32, tag="rinv", name="rinv",
                                        bufs=3)
                        nc.vector.reciprocal_approx_fast(out=rinv, in_=rms)
                        rinv_r = tmp.tile([1, KC], F32R, tag="rinvr",
                                          name="rinvr", bufs=5)
                        nc.vector.tensor_copy(out=rinv_r, in_=rinv)
                        outs2.append((vb, rinv_r))
                    q_bot.append(lambda kc=kc, outs2=outs2: bot_f(kc, outs2))

                def bot_f(kc, outs2):
                    for g in range(G):
                        vb, rinv_r = outs2[g]
                        bc = pbc.tile([128, KC], F32, tag="bc", name="bc")
                        nc.tensor.matmul(bc, lhsT=gk_sb, rhs=rinv_r,
                                         start=True, stop=True)
                        nc.vector.tensor_tensor(
                            out=ktn[g][:, kc * KC:(kc + 1) * KC],
                            in0=vb, in1=bc, op=MULT)

                q_top.append(top_f)
            while q_top or q_mid or q_bot:
                pop(q_top)
                pop(q_mid)
                pop(q_bot)

        # ------------- phase 2: Q projection (own 512 queries) ------------
        with ExitStack() as p2:
            xqp = p2.enter_context(tc.tile_pool(name="xqp", bufs=1))
            wqp = p2.enter_context(tc.tile_pool(name="wqs", bufs=3))
            tmp2 = p2.enter_context(tc.tile_pool(name="tmp2", bufs=3))
            pq = p2.enter_context(tc.tile_pool(name="pq", bufs=2, space="PSUM"))
            pssq2 = p2.enter_context(tc.tile_pool(name="pssq2", bufs=2,
                                                  space="PSUM"))
            pbcq = p2.enter_context(tc.tile_pool(name="pbcq", bufs=2,
                                                 space="PSUM"))
            xq = []
            for et in range(ET):
                xt = xqp.tile([128, QPC], BF16, tag=f"xq{et}", name=f"xq{et}")
                nc.sync.dma_start(out=xt,
                                  in_=xq_d[et * 128:(et + 1) * 128, :])
                xq.append(xt)

            q_top, q_mid, q_bot = [], [], []
            for qc in range(NH):
                pop(q_top)
                wq_sb = wqp.tile([128, ET * 128], BF16, tag="wq", name="wq")
                nc.sync.dma_start(out=wq_sb, in_=wq_d[:, qc, :])
                acc = pq.tile([128, QPC], F32, tag="qacc", name="qacc")
                for et in range(ET):
                    nc.tensor.matmul(acc,
                                     lhsT=wq_sb[:, et * 128:(et + 1) * 128],
                                     rhs=xq[et],
                                     start=(et == 0), stop=(et == ET - 1))
                pop(q_mid)
                pop(q_bot)

                def top_f(qc=qc, acc=acc):
                    sq = tmp2.tile([128, QPC], F32R, tag="sq", name="qsq")
                    nc.scalar.activation(out=sq, in_=acc, func=SQUARE,
                                         bias=bq_sb[:, qc:qc + 1])
                    vb = tmp2.tile([128, QPC], F32, tag="vb", name="qvb",
                                   bufs=4)
                    nc.vector.tensor_scalar(
                        out=vb, in0=acc, scalar1=bq_sb[:, qc:qc + 1],
                        scalar2=None, op0=ADD)

                    def mid_f(qc=qc, sq=sq, vb=vb):
                        ssq = pssq2.tile([1, QPC], F32, tag="ssq",
                                         name="qssq")
                        nc.tensor.matmul(ssq, lhsT=ones_col_r, rhs=sq,
                                         start=True, stop=True)
                        rms = tmp2.tile([1, QPC], F32, tag="rms", name="qrms")
                        nc.scalar.activation(out=rms, in_=ssq, func=SQRT,
                                             scale=1.0 / D, bias=eps_t[:, :])
                        rinv = tmp2.tile([1, QPC], F32, tag="rinv",
                                         name="qrinv", bufs=3)
                        nc.vector.reciprocal_approx_fast(out=rinv, in_=rms)
                        rinv_r = tmp2.tile([1, QPC], F32R, tag="rinvr",
                                           name="qrinvr", bufs=4)
                        nc.vector.tensor_copy(out=rinv_r, in_=rinv)

                        def bot_f(qc=qc, vb=vb, rinv_r=rinv_r):
                            bc = pbcq.tile([128, QPC], F32, tag="bc",
                                           name="qbc")
                            nc.tensor.matmul(bc, lhsT=gq_sb, rhs=rinv_r,
                                             start=True, stop=True)
                            nc.vector.tensor_tensor(out=qtn[:, qc, :],
                                                    in0=vb, in1=bc, op=MULT)
                        q_bot.append(bot_f)
                    q_mid.append(mid_f)
                q_top.append(top_f)
            while q_top or q_mid or q_bot:
                pop(q_top)
                pop(q_mid)
                pop(q_bot)

        # ---------- phase 3: causal attention + phase 4: out proj ---------
        with ExitStack() as p34:
            ptp = p34.enter_context(tc.tile_pool(name="pt", bufs=5))
            tmp3 = p34.enter_context(tc.tile_pool(name="tmp3", bufs=3))
            wop = p34.enter_context(tc.tile_pool(name="wos", bufs=3))
            osb = p34.enter_context(tc.tile_pool(name="osb", bufs=3))
            psc = p34.enter_context(tc.tile_pool(name="psc", bufs=4,
                                                 space="PSUM"))
            pden = p34.enter_context(tc.tile_pool(name="pden", bufs=2,
                                                  space="PSUM"))
            pcx = p34.enter_context(tc.tile_pool(name="pcx", bufs=2,
                                                 space="PSUM"))
            pend3 = []
            pendH = []

            def post_sc(h, g, pair, sc, den, cx):
                used = 256 if pair == (8,) else 512
                pt = ptp.tile([128, QPC], BF16, tag="pt", name="pt")
                nc.scalar.activation(out=pt[:, 0:used], in_=sc[:, 0:used],
                                     func=EXP, scale=SCALE)
                offs = []
                for m, kt in enumerate(pair):
                    off = 0 if m == 0 else 512 - 32 * pair[0]
                    offs.append(off)
                    nc.vector.tensor_tensor(out=pt[:, off:off + 32],
                                            in0=pt[:, off:off + 32],
                                            in1=dmask, op=MULT)
                for m, kt in enumerate(pair):
                    off = offs[m]
                    w = 512 - 32 * kt
                    first = (pair == (0,))
                    last = (pair == (8,))
                    nc.tensor.matmul(den[:, 32 * kt:512], lhsT=ones_col_bf,
                                     rhs=pt[:, off:off + w],
                                     start=first, stop=last)
                    nc.tensor.matmul(cx[:, 32 * kt:512],
                                     lhsT=vtok[:, kt, g * 128:(g + 1) * 128],
                                     rhs=pt[:, off:off + w],
                                     start=first, stop=last)

            def post_head(h, den, cx):
                rd = tmp3.tile([1, QPC], F32, tag="rd", name="rd")
                nc.vector.reciprocal_approx_fast(out=rd, in_=den)
                rd_r = tmp3.tile([1, QPC], F32R, tag="rdr", name="rdr")
                nc.vector.tensor_copy(out=rd_r, in_=rd)
                bc2 = psc.tile([128, QPC], F32, tag="sc", name="bc2")
                nc.tensor.matmul(bc2, lhsT=ones_row_r, rhs=rd_r,
                                 start=True, stop=True)
                bc2s = tmp3.tile([128, QPC], F32, tag="bc2s", name="bc2s")
                nc.vector.tensor_copy(out=bc2s, in_=bc2)
                nc.vector.tensor_tensor(out=ctxt[:, h, :], in0=cx, in1=bc2s,
                                        op=MULT)

            for h in range(NH):
                g = h // GS
                den = pden.tile([1, QPC], F32, tag="den", name="den")
                cx = pcx.tile([128, QPC], F32, tag="cx", name="cx")
                for ti, pair in enumerate(SC_TILES):
                    while len(pend3) > 2:
                        pend3.pop(0)()
                    if ti == 4 and pendH:
                        pendH.pop(0)()
                    sc = psc.tile([128, QPC], F32, tag="sc", name="sc")
                    for m, kt in enumerate(pair):
                        off = 0 if m == 0 else 512 - 32 * pair[0]
                        w = 512 - 32 * kt
                        nc.tensor.matmul(
                            sc[:, off:off + w],
                            lhsT=ktn[g][:, kt * 128:(kt + 1) * 128],
                            rhs=qtn[:, h, 32 * kt:512],
                            start=True, stop=True)
                    pend3.append(
                        lambda h=h, g=g, pair=pair, sc=sc, den=den, cx=cx:
                        post_sc(h, g, pair, sc, den, cx))
                pendH.append(lambda h=h, den=den, cx=cx: post_head(h, den, cx))
            while pend3:
                pend3.pop(0)()
            while pendH:
                pendH.pop(0)()

            # ------------------------ phase 4: out proj -------------------
            pend4 = []
            for c2 in range(ET):
                while len(pend4) > 1:
                    pend4.pop(0)()
                wo_sb = wop.tile([128, ET * 128], BF16, tag="wo", name="wo")
                nc.sync.dma_start(out=wo_sb, in_=wo_d[:, c2, :])
                acc = pcx.tile([128, QPC], F32, tag="cx", name="oacc")
                for ct in range(ET):
                    nc.tensor.matmul(acc,
                                     lhsT=wo_sb[:, ct * 128:(ct + 1) * 128],
                                     rhs=ctxt[:, ct, :],
                                     start=(ct == 0), stop=(ct == ET - 1))

                def post_o(c2=c2, acc=acc):
                    ot = osb.tile([128, QPC], F32, tag="ot", name="ot")
                    nc.vector.tensor_scalar(
                        out=ot, in0=acc, scalar1=bo_sb[:, c2:c2 + 1],
                        scalar2=None, op0=ADD)
                    nc.sync.dma_start(
                        out=out_d[c2 * 128:(c2 + 1) * 128, :], in_=ot)
                pend4.append(post_o)
            while pend4:
                pend4.pop(0)()
    nc.compile()
    return nc


# ---------------------------------------------------------------------------
# host-side sharding
# ---------------------------------------------------------------------------

def make_in_maps(cfg, inputs):
    B, S, E, D, G = cfg["B"], cfg["S"], cfg["E"], cfg["D"], cfg["G"]
    NH, ET, NKT, QPC, GS = derived(cfg)
    x = np.asarray(inputs["x"], np.float32)
    Wq = np.asarray(inputs["Wq"], np.float32)
    Wk = np.asarray(inputs["Wk"], np.float32)
    Wv = np.asarray(inputs["Wv"], np.float32)
    Wo = np.asarray(inputs["Wo"], np.float32)

    wqp = np.ascontiguousarray(
        Wq.reshape(ET, 128, NH, 128).transpose(1, 2, 0, 3)
        .reshape(128, NH, ET * 128).astype(BF))
    wop = np.ascontiguousarray(
        Wo.reshape(ET, 128, ET, 128).transpose(1, 2, 0, 3)
        .reshape(128, ET, ET * 128).astype(BF))
    wkp = np.ascontiguousarray(
        Wk.reshape(ET, 128, G * 128).transpose(1, 0, 2)
        .reshape(128, ET * G * 128).astype(BF))
    wvp = np.ascontiguousarray(
        Wv.reshape(ET, 128, G * 128).transpose(1, 0, 2)
        .reshape(128, ET * G * 128).astype(BF))

    shared = dict(
        WqP=wqp, WoP=wop, WkP=wkp, WvP=wvp,
        bq_t=np.ascontiguousarray(
            np.asarray(inputs["bq"], np.float32).reshape(NH, 128).T),
        bk_t=np.ascontiguousarray(
            np.asarray(inputs["bk"], np.float32).reshape(G, 128).T),
        bv_r=np.ascontiguousarray(
            np.asarray(inputs["bv"], np.float32).reshape(1, G * 128)
            .astype(BF)),
        bo_t=np.ascontiguousarray(
            np.asarray(inputs["bo"], np.float32).reshape(ET, 128).T),
        gq_r=np.ascontiguousarray(
            np.asarray(inputs["gamma_q"], np.float32).reshape(1, 128)),
        gk_r=np.ascontiguousarray(
            np.asarray(inputs["gamma_k"], np.float32).reshape(1, 128)),
        ones_c=np.ones((128, 1), np.float32),
        ones_r=np.ones((1, 128), np.float32),
    )
    xTb = [np.ascontiguousarray(x[b].T.astype(BF)) for b in range(B)]
    in_maps, perms = [], []
    for c in range(8):
        b, j = c // 4, c % 4
        kk = np.arange(128)[:, None]
        ii = np.arange(32)[None, :]
        dmask = (kk <= 4 * ii + j).astype(BF)
        m = dict(shared)
        m["xT"] = xTb[b]
        m["xq"] = np.ascontiguousarray(xTb[b][:, j::4])
        m["dmask"] = np.ascontiguousarray(dmask)
        in_maps.append(m)
        perms.append(j)
    return in_maps, perms


def assemble(cfg, results, perms):
    B, S, E = cfg["B"], cfg["S"], cfg["E"]
    out = np.empty((B, S, E), np.float32)
    for c in range(8):
        b, j = c // 4, perms[c]
        out[b, j::4, :] = results[c]["outT"].T
    return out


_CACHE = {}


def kernel(**inputs):
    cfg = full_cfg()
    if "nc" not in _CACHE:
        _CACHE["nc"] = build_program(cfg)
    nc = _CACHE["nc"]
    in_maps, perms = make_in_maps(cfg, inputs)
    res = run_bass_kernel_spmd(nc, in_maps, list(range(8)))
    return assemble(cfg, res.results, perms)


# revision 9
# speedup vs baseline: 1.8185x; 1.0022x over previous
"""GQA attention block (RMSNorm-QK, causal, GQA) on 8 trn2 NeuronCores — v2.

Sharding: batch over groups of 4 cores; stride-4 query interleave within a
batch. Core c handles batch c//4 and query tokens {j, j+4, ..., j+2044}
(j = c%4), so the causal structure is IDENTICAL on every core: for key tile
kt (128 keys), query columns < 32*kt are fully masked (skipped entirely),
columns [32kt, 32kt+32) are diagonal (one shared [128,32] 0/1 mask), and
the rest are fully valid. Scores / exp / denominator / AV all run on the
causally-valid suffix [32kt, 512) only — ~47% less attention work than the
full rectangle, with zero collectives and one uniform SPMD program.

All matmuls are bf16 (1 cycle/row at any free size on the PE; error budget
2e-2 >> bf16's ~1e-3). Activations are feature-major ("T layout"); V is
projected token-major directly (x-tiles stationary), so the kernel needs no
transposes at all. Partition-dim reductions (RMS sum-of-squares, softmax
denominators) and per-token broadcasts are rank-1 matmuls; reciprocals use
the fast custom-DVE op (~18 bits, ~5x faster than nc.vector.reciprocal).
Score tiles for key-tile pairs (p, 16-p) pack into a single PSUM bank so
exp runs as one activation per bank. Softmax needs no max subtraction:
RMS-normalized q,k bound |scores|/sqrt(D) <= sqrt(D).
"""

import math
import numpy as np
from contextlib import ExitStack

import ml_dtypes
import concourse.bass as bass
import concourse.mybir as mybir
import concourse.tile as tile
from concourse import bacc
from concourse.bass_utils import run_bass_kernel_spmd

F32 = mybir.dt.float32
F32R = mybir.dt.float32r
BF16 = mybir.dt.bfloat16
ADD = mybir.AluOpType.add
MULT = mybir.AluOpType.mult
EXP = mybir.ActivationFunctionType.Exp
SQRT = mybir.ActivationFunctionType.Sqrt
SQUARE = mybir.ActivationFunctionType.Square

BF = ml_dtypes.bfloat16
EPS = 1e-8


def full_cfg():
    return dict(B=2, S=2048, E=2048, D=128, G=2)


def derived(cfg):
    B, S, E, D, G = cfg["B"], cfg["S"], cfg["E"], cfg["D"], cfg["G"]
    NH = E // D            # 16 query heads == E blocks of 128
    ET = E // 128          # 16 contraction tiles of E
    NKT = S // 128         # 16 key tiles
    QPC = S // 4           # 512 queries per core (stride-4 stripe)
    GS = NH // G           # 8 heads per kv group
    assert D == 128 and QPC == 512
    return NH, ET, NKT, QPC, GS


# key-tile pairs pack into one PSUM bank: widths (512-32p) + 32p = 512;
# two pairs pack into one 2-bank [128,1024] mega tile so exp runs as a
# single activation per mega tile (5 ACT calls/head instead of 16).
SC_GROUPS = [
    [(0,), (1, 15)],
    [(2, 14), (3, 13)],
    [(4, 12), (5, 11)],
    [(6, 10), (7, 9)],
    [(8,)],
]


def build_program(cfg):
    B, S, E, D, G = cfg["B"], cfg["S"], cfg["E"], cfg["D"], cfg["G"]
    NH, ET, NKT, QPC, GS = derived(cfg)
    SCALE = 1.0 / math.sqrt(D)
    KC = 512
    NKC = S // KC

    nc = bacc.Bacc()
    xT_d = nc.dram_tensor("xT", [E, S], BF16, kind="ExternalInput")
    xq_d = nc.dram_tensor("xq", [E, QPC], BF16, kind="ExternalInput")
    wq_d = nc.dram_tensor("WqP", [128, NH, ET * 128], BF16, kind="ExternalInput")
    wo_d = nc.dram_tensor("WoP", [128, ET, ET * 128], BF16, kind="ExternalInput")
    wk_d = nc.dram_tensor("WkP", [128, ET * G * 128], BF16, kind="ExternalInput")
    wv_d = nc.dram_tensor("WvP", [128, ET * G * 128], BF16, kind="ExternalInput")
    bq_d = nc.dram_tensor("bq_t", [128, NH], F32, kind="ExternalInput")
    bk_d = nc.dram_tensor("bk_t", [128, G], F32, kind="ExternalInput")
    bv_d = nc.dram_tensor("bv_r", [1, G * 128], BF16, kind="ExternalInput")
    bo_d = nc.dram_tensor("bo_t", [128, ET], F32, kind="ExternalInput")
    gq_d = nc.dram_tensor("gq_r", [1, 128], F32, kind="ExternalInput")
    gk_d = nc.dram_tensor("gk_r", [1, 128], F32, kind="ExternalInput")
    dm_d = nc.dram_tensor("dmask", [128, 32], BF16, kind="ExternalInput")
    out_d = nc.dram_tensor("outT", [E, QPC], F32, kind="ExternalOutput")

    def r(ap):
        return ap if ap.dtype == F32R else ap.bitcast(F32R)

    xT_r = xT_d.rearrange("(t p) s -> p t s", p=128)    # [128, ET, S]
    xq_r = xq_d.rearrange("(t p) q -> p t q", p=128)    # [128, ET, QPC]

    with tile.TileContext(nc) as tc, ExitStack() as top:
        consts = top.enter_context(tc.tile_pool(name="consts", bufs=1))
        persist = top.enter_context(tc.tile_pool(name="persist", bufs=1))
        xqp = top.enter_context(tc.tile_pool(name="xqp", bufs=1))
        wqp = top.enter_context(tc.tile_pool(name="wqs", bufs=3))

        ktn = [persist.tile([128, S], BF16, tag=f"ktn{g}", name=f"ktn{g}")
               for g in range(G)]
        vtok = persist.tile([128, NKT, G * 128], BF16, tag="vtok")
        qtn = persist.tile([128, NH, QPC], BF16, tag="qtn")
        ctxt = persist.tile([128, ET, QPC], BF16, tag="ctxt")

        # ------------- phase 1: K/V projection over all tokens ------------
        with ExitStack() as p1:
            xsp = p1.enter_context(tc.tile_pool(name="xs", bufs=3))
            wkvp = p1.enter_context(tc.tile_pool(name="wkv", bufs=1))
            tmp = p1.enter_context(tc.tile_pool(name="tmp1", bufs=3))

            # startup DMA order: compute-gating transfers first (wk, first
            # x chunk, wv), then prefetches, then consts (needed only once
            # per-chunk post-processing starts ~20us in).
            wk_sb = wkvp.tile([128, ET * G * 128], BF16, tag="wk")
            for i in range(4):
                nc.sync.dma_start(out=wk_sb[:, i * 1024:(i + 1) * 1024],
                                  in_=wk_d[:, i * 1024:(i + 1) * 1024])
            xts = []
            xt0 = xsp.tile([128, ET, KC], BF16, tag="xt", name="xt0")
            for i in range(4):
                nc.sync.dma_start(out=xt0[:, i * 4:(i + 1) * 4, :],
                                  in_=xT_r[:, i * 4:(i + 1) * 4, 0:KC])
            xts.append(xt0)
            wv_sb = wkvp.tile([128, ET * G * 128], BF16, tag="wv")
            for i in range(4):
                nc.sync.dma_start(out=wv_sb[:, i * 1024:(i + 1) * 1024],
                                  in_=wv_d[:, i * 1024:(i + 1) * 1024])
            xt1 = xsp.tile([128, ET, KC], BF16, tag="xt", name="xt1")
            nc.sync.dma_start(out=xt1, in_=xT_r[:, :, KC:2 * KC])
            xts.append(xt1)
            xq_sb = xqp.tile([128, ET, QPC], BF16, tag="xq")
            nc.sync.dma_start(out=xq_sb, in_=xq_r)

            ones_col_bf = consts.tile([128, 1], BF16)
            nc.vector.memset(ones_col_bf, 1.0)
            ones_row_bf = consts.tile([1, 128], BF16)
            nc.vector.memset(ones_row_bf, 1.0)
            eps_t = consts.tile([1, 1], F32)
            nc.vector.memset(eps_t, EPS)
            gq_sb = consts.tile([1, 128], F32R)
            nc.sync.dma_start(out=gq_sb, in_=gq_d[:, :].bitcast(F32R))
            gk_sb = consts.tile([1, 128], F32R)
            nc.sync.dma_start(out=gk_sb, in_=gk_d[:, :].bitcast(F32R))
            bq_sb = consts.tile([128, NH], F32)
            nc.sync.dma_start(out=bq_sb, in_=bq_d[:, :])
            bk_sb = consts.tile([128, G], F32)
            nc.sync.dma_start(out=bk_sb, in_=bk_d[:, :])
            bv_sb = consts.tile([1, G * 128], BF16)
            nc.sync.dma_start(out=bv_sb, in_=bv_d[:, :])
            bo_sb = consts.tile([128, ET], F32)
            nc.sync.dma_start(out=bo_sb, in_=bo_d[:, :])
            dmask = consts.tile([128, 32], BF16)
            nc.sync.dma_start(out=dmask, in_=dm_d[:, :])
            wq_tiles = []
            for qc in range(2):
                wq_sb = wqp.tile([128, ET * 128], BF16, tag="wq", name="wq")
                nc.sync.dma_start(out=wq_sb, in_=wq_d[:, qc, :])
                wq_tiles.append(wq_sb)
            pk = p1.enter_context(tc.tile_pool(name="pk", bufs=1, space="PSUM"))
            pv = p1.enter_context(tc.tile_pool(name="pv", bufs=1, space="PSUM"))
            pssq = p1.enter_context(tc.tile_pool(name="pssq", bufs=2, space="PSUM"))
            pbc = p1.enter_context(tc.tile_pool(name="pbc", bufs=2, space="PSUM"))

            q_top, q_mid, q_bot = [], [], []

            def pop(q):
                if q:
                    q.pop(0)()

            for kc in range(NKC):
                if kc + 2 < NKC:
                    xt = xsp.tile([128, ET, KC], BF16, tag="xt", name="xt")
                    nc.sync.dma_start(
                        out=xt, in_=xT_r[:, :, (kc + 2) * KC:(kc + 3) * KC])
                    xts.append(xt)
                xc = xts[kc]
                pop(q_top)
                acck = pk.tile([128, G, KC], F32, tag="acck", name="acck")
                for g in range(G):
                    for et in range(ET):
                        nc.tensor.matmul(
                            acck[:, g, :],
                            lhsT=wk_sb[:, et * 256 + g * 128:
                                       et * 256 + (g + 1) * 128],
                            rhs=xc[:, et, :], start=(et == 0),
                            stop=(et == ET - 1))
                pop(q_mid)
                accv = pv.tile([128, 4, G * 128], F32, tag="accv", name="accv")
                for s in range(4):
                    for et in range(ET):
                        nc.tensor.matmul(
                            accv[:, s, :],
                            lhsT=xc[:, et, s * 128:(s + 1) * 128],
                            rhs=wv_sb[:, et * 256:(et + 1) * 256],
                            start=(et == 0), stop=False)
                    nc.tensor.matmul(accv[:, s, :], lhsT=ones_row_bf,
                                     rhs=bv_sb, start=False, stop=True)
                pop(q_bot)

                def top_f(kc=kc, acck=acck, accv=accv):
                    outs = []
                    for g in range(G):
                        sq = tmp.tile([128, KC], BF16, tag="sq", name="sq",
                                      bufs=3)
                        nc.scalar.activation(out=sq, in_=acck[:, g, :],
                                             func=SQUARE,
                                             bias=bk_sb[:, g:g + 1])
                        vb = tmp.tile([128, KC], F32, tag="vb", name="vb",
                                      bufs=5)
                        nc.vector.tensor_scalar(
                            out=vb, in0=acck[:, g, :],
                            scalar1=bk_sb[:, g:g + 1], scalar2=None, op0=ADD)
                        outs.append((sq, vb))
                    for s in range(4):
                        nc.scalar.copy(out=vtok[:, kc * 4 + s, :],
                                       in_=accv[:, s, :])
                    q_mid.append(lambda kc=kc, outs=outs: mid_f(kc, outs))

                def mid_f(kc, outs):
                    outs2 = []
                    for g in range(G):
                        sq, vb = outs[g]
                        ssq = pssq.tile([1, KC], F32, tag="ssq", name="ssq")
                        nc.tensor.matmul(ssq, lhsT=ones_col_bf, rhs=sq,
                                         start=True, stop=True)
                        rms = tmp.tile([1, KC], F32, tag="rms", name="rms",
                                       bufs=3)
                        nc.scalar.activation(out=rms, in_=ssq, func=SQRT,
                                             scale=1.0 / D, bias=eps_t[:, :])
                        rinv = tmp.tile([1, KC], F32, tag="rinv", name="rinv",
                                        bufs=3)
                        nc.vector.reciprocal_approx_fast(out=rinv, in_=rms)
                        rinv_r = tmp.tile([1, KC], F32R, tag="rinvr",
                                          name="rinvr", bufs=5)
                        nc.vector.tensor_copy(out=rinv_r, in_=rinv)
                        outs2.append((vb, rinv_r))
                    q_bot.append(lambda kc=kc, outs2=outs2: bot_f(kc, outs2))

                def bot_f(kc, outs2):
                    for g in range(G):
                        vb, rinv_r = outs2[g]
                        bc = pbc.tile([128, KC], F32, tag="bc", name="bc")
                        nc.tensor.matmul(bc, lhsT=gk_sb, rhs=rinv_r,
                                         start=True, stop=True)
                        nc.vector.tensor_tensor(
                            out=ktn[g][:, kc * KC:(kc + 1) * KC],
                            in0=vb, in1=bc, op=MULT)

                q_top.append(top_f)
            while q_top or q_mid or q_bot:
                pop(q_top)
                pop(q_mid)
                pop(q_bot)

        # ------------- phase 2: Q projection (own 512 queries) ------------
        with ExitStack() as p2:
            tmp2 = p2.enter_context(tc.tile_pool(name="tmp2", bufs=3))
            pq = p2.enter_context(tc.tile_pool(name="pq", bufs=2, space="PSUM"))
            pssq2 = p2.enter_context(tc.tile_pool(name="pssq2", bufs=2,
                                                  space="PSUM"))
            pbcq = p2.enter_context(tc.tile_pool(name="pbcq", bufs=2,
                                                 space="PSUM"))
            q_top, q_mid, q_bot = [], [], []
            for qc in range(NH):
                if qc + 2 < NH:
                    wq_sb = wqp.tile([128, ET * 128], BF16, tag="wq",
                                     name="wq")
                    nc.sync.dma_start(out=wq_sb, in_=wq_d[:, qc + 2, :])
                    wq_tiles.append(wq_sb)
                pop(q_top)
                acc = pq.tile([128, QPC], F32, tag="qacc", name="qacc")
                for et in range(ET):
                    nc.tensor.matmul(
                        acc,
                        lhsT=wq_tiles[qc][:, et * 128:(et + 1) * 128],
                        rhs=xq_sb[:, et, :],
                        start=(et == 0), stop=(et == ET - 1))
                pop(q_mid)
                pop(q_bot)

                def top_f(qc=qc, acc=acc):
                    sq = tmp2.tile([128, QPC], BF16, tag="sq", name="qsq")
                    nc.scalar.activation(out=sq, in_=acc, func=SQUARE,
                                         bias=bq_sb[:, qc:qc + 1])
                    vb = tmp2.tile([128, QPC], F32, tag="vb", name="qvb",
                                   bufs=4)
                    nc.vector.tensor_scalar(
                        out=vb, in0=acc, scalar1=bq_sb[:, qc:qc + 1],
                        scalar2=None, op0=ADD)

                    def mid_f(qc=qc, sq=sq, vb=vb):
                        ssq = pssq2.tile([1, QPC], F32, tag="ssq",
                                         name="qssq")
                        nc.tensor.matmul(ssq, lhsT=ones_col_bf, rhs=sq,
                                         start=True, stop=True)
                        rms = tmp2.tile([1, QPC], F32, tag="rms", name="qrms")
                        nc.scalar.activation(out=rms, in_=ssq, func=SQRT,
                                             scale=1.0 / D, bias=eps_t[:, :])
                        rinv = tmp2.tile([1, QPC], F32, tag="rinv",
                                         name="qrinv", bufs=3)
                        nc.vector.reciprocal_approx_fast(out=rinv, in_=rms)
                        rinv_r = tmp2.tile([1, QPC], F32R, tag="rinvr",
                                           name="qrinvr", bufs=4)
                        nc.vector.tensor_copy(out=rinv_r, in_=rinv)

                        def bot_f(qc=qc, vb=vb, rinv_r=rinv_r):
                            bc = pbcq.tile([128, QPC], F32, tag="bc",
                                           name="qbc")
                            nc.tensor.matmul(bc, lhsT=gq_sb, rhs=rinv_r,
                                             start=True, stop=True)
                            nc.vector.tensor_tensor(out=qtn[:, qc, :],
                                                    in0=vb, in1=bc, op=MULT)
                        q_bot.append(bot_f)
                    q_mid.append(mid_f)
                q_top.append(top_f)
            while q_top or q_mid or q_bot:
                pop(q_top)
                pop(q_mid)
                pop(q_bot)

        # ---------- phase 3: causal attention + phase 4: out proj ---------
        with ExitStack() as p34:
            ptp = p34.enter_context(tc.tile_pool(name="pt", bufs=5))
            tmp3 = p34.enter_context(tc.tile_pool(name="tmp3", bufs=3))
            wop = p34.enter_context(tc.tile_pool(name="wos", bufs=3))
            osb = p34.enter_context(tc.tile_pool(name="osb", bufs=3))
            psc = p34.enter_context(tc.tile_pool(name="psc", bufs=2,
                                                 space="PSUM"))
            pden = p34.enter_context(tc.tile_pool(name="pden", bufs=2,
                                                  space="PSUM"))
            pcx = p34.enter_context(tc.tile_pool(name="pcx", bufs=2,
                                                 space="PSUM"))
            pend3 = []
            pendH = []

            def grp_offsets(grp):
                """[(kt, col offset in mega tile, width)] for a group."""
                out = []
                for pi, pair in enumerate(grp):
                    base = 512 * pi
                    for m, kt in enumerate(pair):
                        off = base + (0 if m == 0 else 512 - 32 * pair[0])
                        out.append((kt, off, 512 - 32 * kt))
                return out

            def post_sc(h, g, grp, sc, den, cx):
                used = 256 if len(grp) == 1 else 1024
                pt = ptp.tile([128, 2 * QPC], BF16, tag="pt", name="pt")
                nc.scalar.activation(out=pt[:, 0:used], in_=sc[:, 0:used],
                                     func=EXP, scale=SCALE)
                mem = grp_offsets(grp)
                for kt, off, w in mem:
                    nc.vector.tensor_tensor(out=pt[:, off:off + 32],
                                            in0=pt[:, off:off + 32],
                                            in1=dmask, op=MULT)
                for kt, off, w in mem:
                    first = (kt == 0)
                    last = (kt == 8)
                    nc.tensor.matmul(den[:, 32 * kt:512], lhsT=ones_col_bf,
                                     rhs=pt[:, off:off + w],
                                     start=first, stop=last)
                    nc.tensor.matmul(cx[:, 32 * kt:512],
                                     lhsT=vtok[:, kt, g * 128:(g + 1) * 128],
                                     rhs=pt[:, off:off + w],
                                     start=first, stop=last)

            def post_head(h, den, cx):
                rd = tmp3.tile([1, QPC], F32, tag="rd", name="rd")
                nc.vector.reciprocal_approx_fast(out=rd, in_=den)
                rd_bf = tmp3.tile([1, QPC], BF16, tag="rdbf", name="rdbf")
                nc.vector.tensor_copy(out=rd_bf, in_=rd)
                bc2 = psc.tile([128, 2 * QPC], F32, tag="sc", name="bc2")
                nc.tensor.matmul(bc2[:, 0:QPC], lhsT=ones_row_bf, rhs=rd_bf,
                                 start=True, stop=True)
                bc2s = tmp3.tile([128, QPC], F32, tag="bc2s", name="bc2s")
                nc.vector.tensor_copy(out=bc2s, in_=bc2[:, 0:QPC])
                nc.vector.tensor_tensor(out=ctxt[:, h, :], in0=cx, in1=bc2s,
                                        op=MULT)

            for h in range(NH):
                g = h // GS
                den = pden.tile([1, QPC], F32, tag="den", name="den")
                cx = pcx.tile([128, QPC], F32, tag="cx", name="cx")
                for ti, grp in enumerate(SC_GROUPS):
                    while len(pend3) > 1:
                        pend3.pop(0)()
                    if ti == 2 and pendH:
                        pendH.pop(0)()
                    sc = psc.tile([128, 2 * QPC], F32, tag="sc", name="sc")
                    for kt, off, w in grp_offsets(grp):
                        nc.tensor.matmul(
                            sc[:, off:off + w],
                            lhsT=ktn[g][:, kt * 128:(kt + 1) * 128],
                            rhs=qtn[:, h, 32 * kt:512],
                            start=True, stop=True)
                    pend3.append(
                        lambda h=h, g=g, grp=grp, sc=sc, den=den, cx=cx:
                        post_sc(h, g, grp, sc, den, cx))
                pendH.append(lambda h=h, den=den, cx=cx: post_head(h, den, cx))
            while pend3:
                pend3.pop(0)()
            while pendH:
                pendH.pop(0)()

            # ------------------------ phase 4: out proj -------------------
            pend4 = []
            for c2 in range(ET):
                while len(pend4) > 1:
                    pend4.pop(0)()
                wo_sb = wop.tile([128, ET * 128], BF16, tag="wo", name="wo")
                nc.sync.dma_start(out=wo_sb, in_=wo_d[:, c2, :])
                acc = pcx.tile([128, QPC], F32, tag="cx", name="oacc")
                for ct in range(ET):
                    nc.tensor.matmul(acc,
                                     lhsT=wo_sb[:, ct * 128:(ct + 1) * 128],
                                     rhs=ctxt[:, ct, :],
                                     start=(ct == 0), stop=(ct == ET - 1))

                def post_o(c2=c2, acc=acc):
                    ot = osb.tile([128, QPC], F32, tag="ot", name="ot")
                    nc.vector.tensor_scalar(
                        out=ot, in0=acc, scalar1=bo_sb[:, c2:c2 + 1],
                        scalar2=None, op0=ADD)
                    nc.sync.dma_start(
                        out=out_d[c2 * 128:(c2 + 1) * 128, :], in_=ot)
                pend4.append(post_o)
            while pend4:
                pend4.pop(0)()
    nc.compile()
    return nc


# ---------------------------------------------------------------------------
# host-side sharding
# ---------------------------------------------------------------------------

def make_in_maps(cfg, inputs):
    B, S, E, D, G = cfg["B"], cfg["S"], cfg["E"], cfg["D"], cfg["G"]
    NH, ET, NKT, QPC, GS = derived(cfg)
    x = np.asarray(inputs["x"], np.float32)
    Wq = np.asarray(inputs["Wq"], np.float32)
    Wk = np.asarray(inputs["Wk"], np.float32)
    Wv = np.asarray(inputs["Wv"], np.float32)
    Wo = np.asarray(inputs["Wo"], np.float32)

    wqp = np.ascontiguousarray(
        Wq.reshape(ET, 128, NH, 128).transpose(1, 2, 0, 3)
        .reshape(128, NH, ET * 128).astype(BF))
    wop = np.ascontiguousarray(
        Wo.reshape(ET, 128, ET, 128).transpose(1, 2, 0, 3)
        .reshape(128, ET, ET * 128).astype(BF))
    wkp = np.ascontiguousarray(
        Wk.reshape(ET, 128, G * 128).transpose(1, 0, 2)
        .reshape(128, ET * G * 128).astype(BF))
    wvp = np.ascontiguousarray(
        Wv.reshape(ET, 128, G * 128).transpose(1, 0, 2)
        .reshape(128, ET * G * 128).astype(BF))

    shared = dict(
        WqP=wqp, WoP=wop, WkP=wkp, WvP=wvp,
        bq_t=np.ascontiguousarray(
            np.asarray(inputs["bq"], np.float32).reshape(NH, 128).T),
        bk_t=np.ascontiguousarray(
            np.asarray(inputs["bk"], np.float32).reshape(G, 128).T),
        bv_r=np.ascontiguousarray(
            np.asarray(inputs["bv"], np.float32).reshape(1, G * 128)
            .astype(BF)),
        bo_t=np.ascontiguousarray(
            np.asarray(inputs["bo"], np.float32).reshape(ET, 128).T),
        gq_r=np.ascontiguousarray(
            np.asarray(inputs["gamma_q"], np.float32).reshape(1, 128)),
        gk_r=np.ascontiguousarray(
            np.asarray(inputs["gamma_k"], np.float32).reshape(1, 128)),
    )
    xTb = [np.ascontiguousarray(x[b].T.astype(BF)) for b in range(B)]
    in_maps, perms = [], []
    for c in range(8):
        b, j = c // 4, c % 4
        kk = np.arange(128)[:, None]
        ii = np.arange(32)[None, :]
        dmask = (kk <= 4 * ii + j).astype(BF)
        m = dict(shared)
        m["xT"] = xTb[b]
        m["xq"] = np.ascontiguousarray(xTb[b][:, j::4])
        m["dmask"] = np.ascontiguousarray(dmask)
        in_maps.append(m)
        perms.append(j)
    return in_maps, perms


def assemble(cfg, results, perms):
    B, S, E = cfg["B"], cfg["S"], cfg["E"]
    out = np.empty((B, S, E), np.float32)
    for c in range(8):
        b, j = c // 4, perms[c]
        out[b, j::4, :] = results[c]["outT"].T
    return out


_CACHE = {}


def kernel(**inputs):
    cfg = full_cfg()
    if "nc" not in _CACHE:
        _CACHE["nc"] = build_program(cfg)
    nc = _CACHE["nc"]
    in_maps, perms = make_in_maps(cfg, inputs)
    res = run_bass_kernel_spmd(nc, in_maps, list(range(8)))
    return assemble(cfg, res.results, perms)


# revision 10
# speedup vs baseline: 1.8224x; 1.0021x over previous
"""GQA attention block (RMSNorm-QK, causal, GQA) on 8 trn2 NeuronCores — v2.

Sharding: batch over groups of 4 cores; stride-4 query interleave within a
batch. Core c handles batch c//4 and query tokens {j, j+4, ..., j+2044}
(j = c%4), so the causal structure is IDENTICAL on every core: for key tile
kt (128 keys), query columns < 32*kt are fully masked (skipped entirely),
columns [32kt, 32kt+32) are diagonal (one shared [128,32] 0/1 mask), and
the rest are fully valid. Scores / exp / denominator / AV all run on the
causally-valid suffix [32kt, 512) only — ~47% less attention work than the
full rectangle, with zero collectives and one uniform SPMD program.

All matmuls are bf16 (1 cycle/row at any free size on the PE; error budget
2e-2 >> bf16's ~1e-3). Activations are feature-major ("T layout"); V is
projected token-major directly (x-tiles stationary), so the kernel needs no
transposes at all. Partition-dim reductions (RMS sum-of-squares, softmax
denominators) and per-token broadcasts are rank-1 matmuls; reciprocals use
the fast custom-DVE op (~18 bits, ~5x faster than nc.vector.reciprocal).
Score tiles for key-tile pairs (p, 16-p) pack into a single PSUM bank so
exp runs as one activation per bank. Softmax needs no max subtraction:
RMS-normalized q,k bound |scores|/sqrt(D) <= sqrt(D).
"""

import math
import numpy as np
from contextlib import ExitStack

import ml_dtypes
import concourse.bass as bass
import concourse.mybir as mybir
import concourse.tile as tile
from concourse import bacc
from concourse.bass_utils import run_bass_kernel_spmd

F32 = mybir.dt.float32
F32R = mybir.dt.float32r
BF16 = mybir.dt.bfloat16
ADD = mybir.AluOpType.add
MULT = mybir.AluOpType.mult
EXP = mybir.ActivationFunctionType.Exp
SQRT = mybir.ActivationFunctionType.Sqrt
SQUARE = mybir.ActivationFunctionType.Square

BF = ml_dtypes.bfloat16
EPS = 1e-8


def full_cfg():
    return dict(B=2, S=2048, E=2048, D=128, G=2)


def derived(cfg):
    B, S, E, D, G = cfg["B"], cfg["S"], cfg["E"], cfg["D"], cfg["G"]
    NH = E // D            # 16 query heads == E blocks of 128
    ET = E // 128          # 16 contraction tiles of E
    NKT = S // 128         # 16 key tiles
    QPC = S // 4           # 512 queries per core (stride-4 stripe)
    GS = NH // G           # 8 heads per kv group
    assert D == 128 and QPC == 512
    return NH, ET, NKT, QPC, GS


# key-tile pairs pack into one PSUM bank: widths (512-32p) + 32p = 512;
# two pairs pack into one 2-bank [128,1024] mega tile so exp runs as a
# single activation per mega tile (5 ACT calls/head instead of 16).
SC_GROUPS = [
    [(0,), (1, 15)],
    [(2, 14), (3, 13)],
    [(4, 12), (5, 11)],
    [(6, 10), (7, 9)],
    [(8,)],
]


def build_program(cfg):
    B, S, E, D, G = cfg["B"], cfg["S"], cfg["E"], cfg["D"], cfg["G"]
    NH, ET, NKT, QPC, GS = derived(cfg)
    SCALE = 1.0 / math.sqrt(D)
    KC = 512
    NKC = S // KC

    nc = bacc.Bacc()
    xT_d = nc.dram_tensor("xT", [E, S], BF16, kind="ExternalInput")
    xq_d = nc.dram_tensor("xq", [E, QPC], BF16, kind="ExternalInput")
    wq_d = nc.dram_tensor("WqP", [128, NH, ET * 128], BF16, kind="ExternalInput")
    wo_d = nc.dram_tensor("WoP", [128, ET, ET * 128], BF16, kind="ExternalInput")
    wk_d = nc.dram_tensor("WkP", [128, ET * G * 128], BF16, kind="ExternalInput")
    wv_d = nc.dram_tensor("WvP", [128, ET * G * 128], BF16, kind="ExternalInput")
    bq_d = nc.dram_tensor("bq_t", [128, NH], F32, kind="ExternalInput")
    bk_d = nc.dram_tensor("bk_t", [128, G], F32, kind="ExternalInput")
    bv_d = nc.dram_tensor("bv_r", [1, G * 128], BF16, kind="ExternalInput")
    bo_d = nc.dram_tensor("bo_t", [128, ET], F32, kind="ExternalInput")
    gq_d = nc.dram_tensor("gq_r", [1, 128], F32, kind="ExternalInput")
    gk_d = nc.dram_tensor("gk_r", [1, 128], F32, kind="ExternalInput")
    dm_d = nc.dram_tensor("dmask", [128, 32], BF16, kind="ExternalInput")
    out_d = nc.dram_tensor("outT", [E, QPC], F32, kind="ExternalOutput")

    def r(ap):
        return ap if ap.dtype == F32R else ap.bitcast(F32R)

    xT_r = xT_d.rearrange("(t p) s -> p t s", p=128)    # [128, ET, S]
    xq_r = xq_d.rearrange("(t p) q -> p t q", p=128)    # [128, ET, QPC]

    with tile.TileContext(nc) as tc, ExitStack() as top:
        consts = top.enter_context(tc.tile_pool(name="consts", bufs=1))
        persist = top.enter_context(tc.tile_pool(name="persist", bufs=1))
        xqp = top.enter_context(tc.tile_pool(name="xqp", bufs=1))
        wqp = top.enter_context(tc.tile_pool(name="wqs", bufs=6))

        ktn = [persist.tile([128, S], BF16, tag=f"ktn{g}", name=f"ktn{g}")
               for g in range(G)]
        vtok = persist.tile([128, NKT, G * 128], BF16, tag="vtok")
        qtn = persist.tile([128, NH, QPC], BF16, tag="qtn")
        ctxt = persist.tile([128, ET, QPC], BF16, tag="ctxt")

        # ------------- phase 1: K/V projection over all tokens ------------
        with ExitStack() as p1:
            xsp = p1.enter_context(tc.tile_pool(name="xs", bufs=4))
            wkvp = p1.enter_context(tc.tile_pool(name="wkv", bufs=1))
            tmp = p1.enter_context(tc.tile_pool(name="tmp1", bufs=3))

            # startup DMA order: tiny "starter" transfers first so the
            # first matmuls are gated on ~0.4MB, then the bulk, then
            # prefetches, then consts (needed ~20us in).
            wk_sb = wkvp.tile([128, ET * G * 128], BF16, tag="wk")
            nc.sync.dma_start(out=wk_sb[:, 0:512], in_=wk_d[:, 0:512])
            xts = []
            xt0 = xsp.tile([128, ET, KC], BF16, tag="xt", name="xt0")
            nc.sync.dma_start(out=xt0[:, 0:2, :], in_=xT_r[:, 0:2, 0:KC])
            nc.sync.dma_start(out=wk_sb[:, 512:2048], in_=wk_d[:, 512:2048])
            nc.sync.dma_start(out=xt0[:, 2:8, :], in_=xT_r[:, 2:8, 0:KC])
            nc.sync.dma_start(out=wk_sb[:, 2048:4096],
                              in_=wk_d[:, 2048:4096])
            nc.sync.dma_start(out=xt0[:, 8:16, :], in_=xT_r[:, 8:16, 0:KC])
            xts.append(xt0)
            wv_sb = wkvp.tile([128, ET * G * 128], BF16, tag="wv")
            for i in range(2):
                nc.sync.dma_start(out=wv_sb[:, i * 2048:(i + 1) * 2048],
                                  in_=wv_d[:, i * 2048:(i + 1) * 2048])
            xt1 = xsp.tile([128, ET, KC], BF16, tag="xt", name="xt1")
            nc.sync.dma_start(out=xt1, in_=xT_r[:, :, KC:2 * KC])
            xts.append(xt1)
            xq_sb = xqp.tile([128, ET, QPC], BF16, tag="xq")
            nc.sync.dma_start(out=xq_sb, in_=xq_r)

            ones_col_bf = consts.tile([128, 1], BF16)
            nc.vector.memset(ones_col_bf, 1.0)
            ones_row_bf = consts.tile([1, 128], BF16)
            nc.vector.memset(ones_row_bf, 1.0)
            eps_t = consts.tile([1, 1], F32)
            nc.vector.memset(eps_t, EPS)
            gq_sb = consts.tile([1, 128], F32R)
            nc.sync.dma_start(out=gq_sb, in_=gq_d[:, :].bitcast(F32R))
            gk_sb = consts.tile([1, 128], F32R)
            nc.sync.dma_start(out=gk_sb, in_=gk_d[:, :].bitcast(F32R))
            bq_sb = consts.tile([128, NH], F32)
            nc.sync.dma_start(out=bq_sb, in_=bq_d[:, :])
            bk_sb = consts.tile([128, G], F32)
            nc.sync.dma_start(out=bk_sb, in_=bk_d[:, :])
            bv_sb = consts.tile([1, G * 128], BF16)
            nc.sync.dma_start(out=bv_sb, in_=bv_d[:, :])
            bo_sb = consts.tile([128, ET], F32)
            nc.sync.dma_start(out=bo_sb, in_=bo_d[:, :])
            dmask = consts.tile([128, 32], BF16)
            nc.sync.dma_start(out=dmask, in_=dm_d[:, :])
            wq_tiles = []
            for qc in range(2):
                wq_sb = wqp.tile([128, ET * 128], BF16, tag="wq", name="wq")
                nc.sync.dma_start(out=wq_sb, in_=wq_d[:, qc, :])
                wq_tiles.append(wq_sb)
            pk = p1.enter_context(tc.tile_pool(name="pk", bufs=1, space="PSUM"))
            pv = p1.enter_context(tc.tile_pool(name="pv", bufs=1, space="PSUM"))
            pssq = p1.enter_context(tc.tile_pool(name="pssq", bufs=2, space="PSUM"))
            pbc = p1.enter_context(tc.tile_pool(name="pbc", bufs=2, space="PSUM"))

            q_top, q_mid, q_bot = [], [], []

            def pop(q):
                if q:
                    q.pop(0)()

            for kc in range(NKC):
                if kc + 2 < NKC:
                    xt = xsp.tile([128, ET, KC], BF16, tag="xt", name="xt")
                    nc.sync.dma_start(
                        out=xt, in_=xT_r[:, :, (kc + 2) * KC:(kc + 3) * KC])
                    xts.append(xt)
                xc = xts[kc]
                pop(q_top)
                acck = pk.tile([128, G, KC], F32, tag="acck", name="acck")
                for g in range(G):
                    for et in range(ET):
                        nc.tensor.matmul(
                            acck[:, g, :],
                            lhsT=wk_sb[:, et * 256 + g * 128:
                                       et * 256 + (g + 1) * 128],
                            rhs=xc[:, et, :], start=(et == 0),
                            stop=(et == ET - 1))
                pop(q_mid)
                accv = pv.tile([128, 4, G * 128], F32, tag="accv", name="accv")
                for s in range(4):
                    for et in range(ET):
                        nc.tensor.matmul(
                            accv[:, s, :],
                            lhsT=xc[:, et, s * 128:(s + 1) * 128],
                            rhs=wv_sb[:, et * 256:(et + 1) * 256],
                            start=(et == 0), stop=False)
                    nc.tensor.matmul(accv[:, s, :], lhsT=ones_row_bf,
                                     rhs=bv_sb, start=False, stop=True)
                pop(q_bot)

                def top_f(kc=kc, acck=acck, accv=accv):
                    outs = []
                    for g in range(G):
                        sq = tmp.tile([128, KC], BF16, tag="sq", name="sq",
                                      bufs=3)
                        nc.scalar.activation(out=sq, in_=acck[:, g, :],
                                             func=SQUARE,
                                             bias=bk_sb[:, g:g + 1])
                        vb = tmp.tile([128, KC], F32, tag="vb", name="vb",
                                      bufs=5)
                        nc.vector.tensor_scalar(
                            out=vb, in0=acck[:, g, :],
                            scalar1=bk_sb[:, g:g + 1], scalar2=None, op0=ADD)
                        outs.append((sq, vb))
                    for s in range(4):
                        nc.scalar.copy(out=vtok[:, kc * 4 + s, :],
                                       in_=accv[:, s, :])
                    q_mid.append(lambda kc=kc, outs=outs: mid_f(kc, outs))

                def mid_f(kc, outs):
                    outs2 = []
                    for g in range(G):
                        sq, vb = outs[g]
                        ssq = pssq.tile([1, KC], F32, tag="ssq", name="ssq")
                        nc.tensor.matmul(ssq, lhsT=ones_col_bf, rhs=sq,
                                         start=True, stop=True)
                        rms = tmp.tile([1, KC], F32, tag="rms", name="rms",
                                       bufs=3)
                        nc.scalar.activation(out=rms, in_=ssq, func=SQRT,
                                             scale=1.0 / D, bias=eps_t[:, :])
                        rinv = tmp.tile([1, KC], F32, tag="rinv", name="rinv",
                                        bufs=3)
                        nc.vector.reciprocal_approx_fast(out=rinv, in_=rms)
                        rinv_r = tmp.tile([1, KC], F32R, tag="rinvr",
                                          name="rinvr", bufs=5)
                        nc.vector.tensor_copy(out=rinv_r, in_=rinv)
                        outs2.append((vb, rinv_r))
                    q_bot.append(lambda kc=kc, outs2=outs2: bot_f(kc, outs2))

                def bot_f(kc, outs2):
                    for g in range(G):
                        vb, rinv_r = outs2[g]
                        bc = pbc.tile([128, KC], F32, tag="bc", name="bc")
                        nc.tensor.matmul(bc, lhsT=gk_sb, rhs=rinv_r,
                                         start=True, stop=True)
                        nc.vector.tensor_tensor(
                            out=ktn[g][:, kc * KC:(kc + 1) * KC],
                            in0=vb, in1=bc, op=MULT)

                q_top.append(top_f)
            while q_top or q_mid or q_bot:
                pop(q_top)
                pop(q_mid)
                pop(q_bot)

        # ------------- phase 2: Q projection (own 512 queries) ------------
        with ExitStack() as p2:
            tmp2 = p2.enter_context(tc.tile_pool(name="tmp2", bufs=3))
            pq = p2.enter_context(tc.tile_pool(name="pq", bufs=2, space="PSUM"))
            pssq2 = p2.enter_context(tc.tile_pool(name="pssq2", bufs=2,
                                                  space="PSUM"))
            pbcq = p2.enter_context(tc.tile_pool(name="pbcq", bufs=2,
                                                 space="PSUM"))
            for qc in range(2, 4):
                wq_sb = wqp.tile([128, ET * 128], BF16, tag="wq", name="wq")
                nc.sync.dma_start(out=wq_sb, in_=wq_d[:, qc, :])
                wq_tiles.append(wq_sb)
            q_top, q_mid, q_bot = [], [], []
            for qc in range(NH):
                if qc + 4 < NH:
                    wq_sb = wqp.tile([128, ET * 128], BF16, tag="wq",
                                     name="wq")
                    nc.sync.dma_start(out=wq_sb, in_=wq_d[:, qc + 4, :])
                    wq_tiles.append(wq_sb)
                pop(q_top)
                acc = pq.tile([128, QPC], F32, tag="qacc", name="qacc")
                for et in range(ET):
                    nc.tensor.matmul(
                        acc,
                        lhsT=wq_tiles[qc][:, et * 128:(et + 1) * 128],
                        rhs=xq_sb[:, et, :],
                        start=(et == 0), stop=(et == ET - 1))
                pop(q_mid)
                pop(q_bot)

                def top_f(qc=qc, acc=acc):
                    sq = tmp2.tile([128, QPC], BF16, tag="sq", name="qsq")
                    nc.scalar.activation(out=sq, in_=acc, func=SQUARE,
                                         bias=bq_sb[:, qc:qc + 1])
                    vb = tmp2.tile([128, QPC], F32, tag="vb", name="qvb",
                                   bufs=4)
                    nc.vector.tensor_scalar(
                        out=vb, in0=acc, scalar1=bq_sb[:, qc:qc + 1],
                        scalar2=None, op0=ADD)

                    def mid_f(qc=qc, sq=sq, vb=vb):
                        ssq = pssq2.tile([1, QPC], F32, tag="ssq",
                                         name="qssq")
                        nc.tensor.matmul(ssq, lhsT=ones_col_bf, rhs=sq,
                                         start=True, stop=True)
                        rms = tmp2.tile([1, QPC], F32, tag="rms", name="qrms")
                        nc.scalar.activation(out=rms, in_=ssq, func=SQRT,
                                             scale=1.0 / D, bias=eps_t[:, :])
                        rinv = tmp2.tile([1, QPC], F32, tag="rinv",
                                         name="qrinv", bufs=3)
                        nc.vector.reciprocal_approx_fast(out=rinv, in_=rms)
                        rinv_r = tmp2.tile([1, QPC], F32R, tag="rinvr",
                                           name="qrinvr", bufs=4)
                        nc.vector.tensor_copy(out=rinv_r, in_=rinv)

                        def bot_f(qc=qc, vb=vb, rinv_r=rinv_r):
                            bc = pbcq.tile([128, QPC], F32, tag="bc",
                                           name="qbc")
                            nc.tensor.matmul(bc, lhsT=gq_sb, rhs=rinv_r,
                                             start=True, stop=True)
                            nc.vector.tensor_tensor(out=qtn[:, qc, :],
                                                    in0=vb, in1=bc, op=MULT)
                        q_bot.append(bot_f)
                    q_mid.append(mid_f)
                q_top.append(top_f)
            while q_top or q_mid or q_bot:
                pop(q_top)
                pop(q_mid)
                pop(q_bot)

        # ---------- phase 3: causal attention + phase 4: out proj ---------
        with ExitStack() as p34:
            ptp = p34.enter_context(tc.tile_pool(name="pt", bufs=5))
            tmp3 = p34.enter_context(tc.tile_pool(name="tmp3", bufs=3))
            wop = p34.enter_context(tc.tile_pool(name="wos", bufs=16))
            osb = p34.enter_context(tc.tile_pool(name="osb", bufs=3))
            psc = p34.enter_context(tc.tile_pool(name="psc", bufs=2,
                                                 space="PSUM"))
            pden = p34.enter_context(tc.tile_pool(name="pden", bufs=2,
                                                  space="PSUM"))
            pcx = p34.enter_context(tc.tile_pool(name="pcx", bufs=2,
                                                 space="PSUM"))
            pend_exp = []
            pend_acc = []
            pendH = []

            def grp_offsets(grp):
                """[(kt, col offset in mega tile, width)] for a group."""
                out = []
                for pi, pair in enumerate(grp):
                    base = 512 * pi
                    for m, kt in enumerate(pair):
                        off = base + (0 if m == 0 else 512 - 32 * pair[0])
                        out.append((kt, off, 512 - 32 * kt))
                return out

            def post_exp(h, g, grp, sc, den, cx):
                used = 256 if len(grp) == 1 else 1024
                pt = ptp.tile([128, 2 * QPC], BF16, tag="pt", name="pt")
                nc.scalar.activation(out=pt[:, 0:used], in_=sc[:, 0:used],
                                     func=EXP, scale=SCALE)
                mem = grp_offsets(grp)
                for kt, off, w in mem:
                    nc.vector.tensor_tensor(out=pt[:, off:off + 32],
                                            in0=pt[:, off:off + 32],
                                            in1=dmask, op=MULT)

                def post_acc():
                    for kt, off, w in mem:
                        first = (kt == 0)
                        last = (kt == 8)
                        nc.tensor.matmul(den[:, 32 * kt:512],
                                         lhsT=ones_col_bf,
                                         rhs=pt[:, off:off + w],
                                         start=first, stop=last)
                        nc.tensor.matmul(
                            cx[:, 32 * kt:512],
                            lhsT=vtok[:, kt, g * 128:(g + 1) * 128],
                            rhs=pt[:, off:off + w],
                            start=first, stop=last)
                pend_acc.append(post_acc)

            def post_head(h, den, cx):
                rd = tmp3.tile([1, QPC], F32, tag="rd", name="rd")
                nc.vector.reciprocal_approx_fast(out=rd, in_=den)
                rd_bf = tmp3.tile([1, QPC], BF16, tag="rdbf", name="rdbf")
                nc.vector.tensor_copy(out=rd_bf, in_=rd)
                bc2 = psc.tile([128, 2 * QPC], F32, tag="sc", name="bc2")
                nc.tensor.matmul(bc2[:, 0:QPC], lhsT=ones_row_bf, rhs=rd_bf,
                                 start=True, stop=True)
                bc2s = tmp3.tile([128, QPC], F32, tag="bc2s", name="bc2s")
                nc.scalar.copy(out=bc2s, in_=bc2[:, 0:QPC])
                nc.vector.tensor_tensor(out=ctxt[:, h, :], in0=cx, in1=bc2s,
                                        op=MULT)

            # prefetch all Wo tiles while Sync is otherwise idle in phase 3
            wo_tiles = []
            for c2 in range(ET):
                wo_sb = wop.tile([128, ET * 128], BF16, tag="wo", name="wo")
                nc.sync.dma_start(out=wo_sb, in_=wo_d[:, c2, :])
                wo_tiles.append(wo_sb)

            for h in range(NH):
                g = h // GS
                den = pden.tile([1, QPC], F32, tag="den", name="den")
                cx = pcx.tile([128, QPC], F32, tag="cx", name="cx")
                for ti, grp in enumerate(SC_GROUPS):
                    if pend_acc:
                        pend_acc.pop(0)()
                    if pend_exp:
                        pend_exp.pop(0)()
                    if ti == 2 and pendH:
                        pendH.pop(0)()
                    sc = psc.tile([128, 2 * QPC], F32, tag="sc", name="sc")
                    for kt, off, w in grp_offsets(grp):
                        nc.tensor.matmul(
                            sc[:, off:off + w],
                            lhsT=ktn[g][:, kt * 128:(kt + 1) * 128],
                            rhs=qtn[:, h, 32 * kt:512],
                            start=True, stop=True)
                    pend_exp.append(
                        lambda h=h, g=g, grp=grp, sc=sc, den=den, cx=cx:
                        post_exp(h, g, grp, sc, den, cx))
                pendH.append(lambda h=h, den=den, cx=cx: post_head(h, den, cx))
            while pend_exp or pend_acc:
                if pend_acc:
                    pend_acc.pop(0)()
                if pend_exp:
                    pend_exp.pop(0)()
            while pendH:
                pendH.pop(0)()

            # ------------------------ phase 4: out proj -------------------
            pend4 = []
            for c2 in range(ET):
                while len(pend4) > 1:
                    pend4.pop(0)()
                wo_sb = wo_tiles[c2]
                acc = pcx.tile([128, QPC], F32, tag="cx", name="oacc")
                for ct in range(ET):
                    nc.tensor.matmul(acc,
                                     lhsT=wo_sb[:, ct * 128:(ct + 1) * 128],
                                     rhs=ctxt[:, ct, :],
                                     start=(ct == 0), stop=(ct == ET - 1))

                def post_o(c2=c2, acc=acc):
                    ot = osb.tile([128, QPC], F32, tag="ot", name="ot")
                    nc.vector.tensor_scalar(
                        out=ot, in0=acc, scalar1=bo_sb[:, c2:c2 + 1],
                        scalar2=None, op0=ADD)
                    nc.sync.dma_start(
                        out=out_d[c2 * 128:(c2 + 1) * 128, :], in_=ot)
                pend4.append(post_o)
            while pend4:
                pend4.pop(0)()
    nc.compile()
    return nc


# ---------------------------------------------------------------------------
# host-side sharding
# ---------------------------------------------------------------------------

def make_in_maps(cfg, inputs):
    B, S, E, D, G = cfg["B"], cfg["S"], cfg["E"], cfg["D"], cfg["G"]
    NH, ET, NKT, QPC, GS = derived(cfg)
    x = np.asarray(inputs["x"], np.float32)
    Wq = np.asarray(inputs["Wq"], np.float32)
    Wk = np.asarray(inputs["Wk"], np.float32)
    Wv = np.asarray(inputs["Wv"], np.float32)
    Wo = np.asarray(inputs["Wo"], np.float32)

    wqp = np.ascontiguousarray(
        Wq.reshape(ET, 128, NH, 128).transpose(1, 2, 0, 3)
        .reshape(128, NH, ET * 128).astype(BF))
    wop = np.ascontiguousarray(
        Wo.reshape(ET, 128, ET, 128).transpose(1, 2, 0, 3)
        .reshape(128, ET, ET * 128).astype(BF))
    wkp = np.ascontiguousarray(
        Wk.reshape(ET, 128, G * 128).transpose(1, 0, 2)
        .reshape(128, ET * G * 128).astype(BF))
    wvp = np.ascontiguousarray(
        Wv.reshape(ET, 128, G * 128).transpose(1, 0, 2)
        .reshape(128, ET * G * 128).astype(BF))

    shared = dict(
        WqP=wqp, WoP=wop, WkP=wkp, WvP=wvp,
        bq_t=np.ascontiguousarray(
            np.asarray(inputs["bq"], np.float32).reshape(NH, 128).T),
        bk_t=np.ascontiguousarray(
            np.asarray(inputs["bk"], np.float32).reshape(G, 128).T),
        bv_r=np.ascontiguousarray(
            np.asarray(inputs["bv"], np.float32).reshape(1, G * 128)
            .astype(BF)),
        bo_t=np.ascontiguousarray(
            np.asarray(inputs["bo"], np.float32).reshape(ET, 128).T),
        gq_r=np.ascontiguousarray(
            np.asarray(inputs["gamma_q"], np.float32).reshape(1, 128)),
        gk_r=np.ascontiguousarray(
            np.asarray(inputs["gamma_k"], np.float32).reshape(1, 128)),
    )
    xTb = [np.ascontiguousarray(x[b].T.astype(BF)) for b in range(B)]
    in_maps, perms = [], []
    for c in range(8):
        b, j = c // 4, c % 4
        kk = np.arange(128)[:, None]
        ii = np.arange(32)[None, :]
        dmask = (kk <= 4 * ii + j).astype(BF)
        m = dict(shared)
        m["xT"] = xTb[b]
        m["xq"] = np.ascontiguousarray(xTb[b][:, j::4])
        m["dmask"] = np.ascontiguousarray(dmask)
        in_maps.append(m)
        perms.append(j)
    return in_maps, perms


def assemble(cfg, results, perms):
    B, S, E = cfg["B"], cfg["S"], cfg["E"]
    out = np.empty((B, S, E), np.float32)
    for c in range(8):
        b, j = c // 4, perms[c]
        out[b, j::4, :] = results[c]["outT"].T
    return out


_CACHE = {}


def kernel(**inputs):
    cfg = full_cfg()
    if "nc" not in _CACHE:
        _CACHE["nc"] = build_program(cfg)
    nc = _CACHE["nc"]
    in_maps, perms = make_in_maps(cfg, inputs)
    res = run_bass_kernel_spmd(nc, in_maps, list(range(8)))
    return assemble(cfg, res.results, perms)


# revision 11
# speedup vs baseline: 1.8381x; 1.0086x over previous
"""GQA attention block (RMSNorm-QK, causal, GQA) on 8 trn2 NeuronCores — v2.

Sharding: batch over groups of 4 cores; stride-4 query interleave within a
batch. Core c handles batch c//4 and query tokens {j, j+4, ..., j+2044}
(j = c%4), so the causal structure is IDENTICAL on every core: for key tile
kt (128 keys), query columns < 32*kt are fully masked (skipped entirely),
columns [32kt, 32kt+32) are diagonal (one shared [128,32] 0/1 mask), and
the rest are fully valid. Scores / exp / denominator / AV all run on the
causally-valid suffix [32kt, 512) only — ~47% less attention work than the
full rectangle, with zero collectives and one uniform SPMD program.

All matmuls are bf16 (1 cycle/row at any free size on the PE; error budget
2e-2 >> bf16's ~1e-3). Activations are feature-major ("T layout"); V is
projected token-major directly (x-tiles stationary), so the kernel needs no
transposes at all. Partition-dim reductions (RMS sum-of-squares, softmax
denominators) and per-token broadcasts are rank-1 matmuls; reciprocals use
the fast custom-DVE op (~18 bits, ~5x faster than nc.vector.reciprocal).
Score tiles for key-tile pairs (p, 16-p) pack into a single PSUM bank so
exp runs as one activation per bank. Softmax needs no max subtraction:
RMS-normalized q,k bound |scores|/sqrt(D) <= sqrt(D).
"""

import math
import numpy as np
from contextlib import ExitStack

import ml_dtypes
import concourse.bass as bass
import concourse.mybir as mybir
import concourse.tile as tile
from concourse import bacc
from concourse.bass_utils import run_bass_kernel_spmd

F32 = mybir.dt.float32
F32R = mybir.dt.float32r
BF16 = mybir.dt.bfloat16
ADD = mybir.AluOpType.add
MULT = mybir.AluOpType.mult
EXP = mybir.ActivationFunctionType.Exp
SQRT = mybir.ActivationFunctionType.Sqrt
SQUARE = mybir.ActivationFunctionType.Square

BF = ml_dtypes.bfloat16
EPS = 1e-8


def full_cfg():
    return dict(B=2, S=2048, E=2048, D=128, G=2)


def derived(cfg):
    B, S, E, D, G = cfg["B"], cfg["S"], cfg["E"], cfg["D"], cfg["G"]
    NH = E // D            # 16 query heads == E blocks of 128
    ET = E // 128          # 16 contraction tiles of E
    NKT = S // 128         # 16 key tiles
    QPC = S // 4           # 512 queries per core (stride-4 stripe)
    GS = NH // G           # 8 heads per kv group
    assert D == 128 and QPC == 512
    return NH, ET, NKT, QPC, GS


# key-tile pairs pack into one PSUM bank: widths (512-32p) + 32p = 512;
# two pairs pack into one 2-bank [128,1024] mega tile so exp runs as a
# single activation per mega tile (5 ACT calls/head instead of 16).
SC_GROUPS = [
    [(0,), (1, 15)],
    [(2, 14), (3, 13)],
    [(4, 12), (5, 11)],
    [(6, 10), (7, 9)],
    [(8,)],
]


def build_program(cfg):
    B, S, E, D, G = cfg["B"], cfg["S"], cfg["E"], cfg["D"], cfg["G"]
    NH, ET, NKT, QPC, GS = derived(cfg)
    SCALE = 1.0 / math.sqrt(D)
    KC = 512
    NKC = S // KC

    nc = bacc.Bacc()
    xT_d = nc.dram_tensor("xT", [E, S], BF16, kind="ExternalInput")
    xq_d = nc.dram_tensor("xq", [E, QPC], BF16, kind="ExternalInput")
    wq_d = nc.dram_tensor("WqP", [128, NH, ET * 128], BF16, kind="ExternalInput")
    wo_d = nc.dram_tensor("WoP", [128, ET, ET * 128], BF16, kind="ExternalInput")
    wk_d = nc.dram_tensor("WkP", [128, ET * G * 128], BF16, kind="ExternalInput")
    wv_d = nc.dram_tensor("WvP", [128, ET * G * 128], BF16, kind="ExternalInput")
    bq_d = nc.dram_tensor("bq_t", [128, NH], F32, kind="ExternalInput")
    bk_d = nc.dram_tensor("bk_t", [128, G], F32, kind="ExternalInput")
    bv_d = nc.dram_tensor("bv_r", [1, G * 128], BF16, kind="ExternalInput")
    bo_d = nc.dram_tensor("bo_t", [128, ET], F32, kind="ExternalInput")
    gq_d = nc.dram_tensor("gq_r", [1, 128], BF16, kind="ExternalInput")
    gk_d = nc.dram_tensor("gk_r", [1, 128], BF16, kind="ExternalInput")
    dm_d = nc.dram_tensor("dmask", [128, 32], BF16, kind="ExternalInput")
    out_d = nc.dram_tensor("outT", [E, QPC], F32, kind="ExternalOutput")

    def r(ap):
        return ap if ap.dtype == F32R else ap.bitcast(F32R)

    xT_r = xT_d.rearrange("(t p) s -> p t s", p=128)    # [128, ET, S]
    xq_r = xq_d.rearrange("(t p) q -> p t q", p=128)    # [128, ET, QPC]

    with tile.TileContext(nc) as tc, ExitStack() as top:
        consts = top.enter_context(tc.tile_pool(name="consts", bufs=1))
        persist = top.enter_context(tc.tile_pool(name="persist", bufs=1))
        xqp = top.enter_context(tc.tile_pool(name="xqp", bufs=1))
        wqp = top.enter_context(tc.tile_pool(name="wqs", bufs=6))

        ktn = [persist.tile([128, S], BF16, tag=f"ktn{g}", name=f"ktn{g}")
               for g in range(G)]
        vtok = persist.tile([128, NKT, G * 128], BF16, tag="vtok")
        qtn = persist.tile([128, NH, QPC], BF16, tag="qtn")
        ctxt = persist.tile([128, ET, QPC], BF16, tag="ctxt")

        # ------------- phase 1: K/V projection over all tokens ------------
        with ExitStack() as p1:
            xsp = p1.enter_context(tc.tile_pool(name="xs", bufs=4))
            wkvp = p1.enter_context(tc.tile_pool(name="wkv", bufs=1))
            tmp = p1.enter_context(tc.tile_pool(name="tmp1", bufs=3))

            # startup DMA order: tiny "starter" transfers first so the
            # first matmuls are gated on ~0.4MB, then the bulk, then
            # prefetches, then consts (needed ~20us in).
            wk_sb = wkvp.tile([128, ET * G * 128], BF16, tag="wk")
            nc.sync.dma_start(out=wk_sb[:, 0:512], in_=wk_d[:, 0:512])
            xts = []
            xt0 = xsp.tile([128, ET, KC], BF16, tag="xt", name="xt0")
            nc.sync.dma_start(out=xt0[:, 0:2, :], in_=xT_r[:, 0:2, 0:KC])
            nc.sync.dma_start(out=wk_sb[:, 512:2048], in_=wk_d[:, 512:2048])
            nc.sync.dma_start(out=xt0[:, 2:8, :], in_=xT_r[:, 2:8, 0:KC])
            nc.sync.dma_start(out=wk_sb[:, 2048:4096],
                              in_=wk_d[:, 2048:4096])
            nc.sync.dma_start(out=xt0[:, 8:16, :], in_=xT_r[:, 8:16, 0:KC])
            xts.append(xt0)
            wv_sb = wkvp.tile([128, ET * G * 128], BF16, tag="wv")
            for i in range(2):
                nc.sync.dma_start(out=wv_sb[:, i * 2048:(i + 1) * 2048],
                                  in_=wv_d[:, i * 2048:(i + 1) * 2048])
            xt1 = xsp.tile([128, ET, KC], BF16, tag="xt", name="xt1")
            nc.sync.dma_start(out=xt1, in_=xT_r[:, :, KC:2 * KC])
            xts.append(xt1)
            xq_sb = xqp.tile([128, ET, QPC], BF16, tag="xq")
            nc.sync.dma_start(out=xq_sb, in_=xq_r)

            ones_col_bf = consts.tile([128, 1], BF16)
            nc.vector.memset(ones_col_bf, 1.0)
            ones_row_bf = consts.tile([1, 128], BF16)
            nc.vector.memset(ones_row_bf, 1.0)
            eps_t = consts.tile([1, 1], F32)
            nc.vector.memset(eps_t, EPS)
            gq_sb = consts.tile([1, 128], BF16)
            nc.sync.dma_start(out=gq_sb, in_=gq_d[:, :])
            gk_sb = consts.tile([1, 128], BF16)
            nc.sync.dma_start(out=gk_sb, in_=gk_d[:, :])
            bq_sb = consts.tile([128, NH], F32)
            nc.sync.dma_start(out=bq_sb, in_=bq_d[:, :])
            bk_sb = consts.tile([128, G], F32)
            nc.sync.dma_start(out=bk_sb, in_=bk_d[:, :])
            bv_sb = consts.tile([1, G * 128], BF16)
            nc.sync.dma_start(out=bv_sb, in_=bv_d[:, :])
            bo_sb = consts.tile([128, ET], F32)
            nc.sync.dma_start(out=bo_sb, in_=bo_d[:, :])
            dmask = consts.tile([128, 32], BF16)
            nc.sync.dma_start(out=dmask, in_=dm_d[:, :])
            wq_tiles = []
            for qc in range(2):
                wq_sb = wqp.tile([128, ET * 128], BF16, tag="wq", name="wq")
                nc.sync.dma_start(out=wq_sb, in_=wq_d[:, qc, :])
                wq_tiles.append(wq_sb)
            pk = p1.enter_context(tc.tile_pool(name="pk", bufs=1, space="PSUM"))
            pv = p1.enter_context(tc.tile_pool(name="pv", bufs=1, space="PSUM"))
            pssq = p1.enter_context(tc.tile_pool(name="pssq", bufs=2, space="PSUM"))
            pbc = p1.enter_context(tc.tile_pool(name="pbc", bufs=2, space="PSUM"))

            q_top, q_mid, q_bot = [], [], []

            def pop(q):
                if q:
                    q.pop(0)()

            for kc in range(NKC):
                if kc + 2 < NKC:
                    xt = xsp.tile([128, ET, KC], BF16, tag="xt", name="xt")
                    nc.sync.dma_start(
                        out=xt, in_=xT_r[:, :, (kc + 2) * KC:(kc + 3) * KC])
                    xts.append(xt)
                xc = xts[kc]
                pop(q_top)
                acck = pk.tile([128, G, KC], F32, tag="acck", name="acck")
                for g in range(G):
                    for et in range(ET):
                        nc.tensor.matmul(
                            acck[:, g, :],
                            lhsT=wk_sb[:, et * 256 + g * 128:
                                       et * 256 + (g + 1) * 128],
                            rhs=xc[:, et, :], start=(et == 0),
                            stop=(et == ET - 1))
                pop(q_mid)
                accv = pv.tile([128, 4, G * 128], F32, tag="accv", name="accv")
                for s in range(4):
                    for et in range(ET):
                        nc.tensor.matmul(
                            accv[:, s, :],
                            lhsT=xc[:, et, s * 128:(s + 1) * 128],
                            rhs=wv_sb[:, et * 256:(et + 1) * 256],
                            start=(et == 0), stop=False)
                    nc.tensor.matmul(accv[:, s, :], lhsT=ones_row_bf,
                                     rhs=bv_sb, start=False, stop=True)
                pop(q_bot)

                def top_f(kc=kc, acck=acck, accv=accv):
                    outs = []
                    for g in range(G):
                        sq = tmp.tile([128, KC], BF16, tag="sq", name="sq",
                                      bufs=3)
                        nc.scalar.activation(out=sq, in_=acck[:, g, :],
                                             func=SQUARE,
                                             bias=bk_sb[:, g:g + 1])
                        vb = tmp.tile([128, KC], F32, tag="vb", name="vb",
                                      bufs=5)
                        nc.vector.tensor_scalar(
                            out=vb, in0=acck[:, g, :],
                            scalar1=bk_sb[:, g:g + 1], scalar2=None, op0=ADD)
                        outs.append((sq, vb))
                    for s in range(4):
                        nc.scalar.copy(out=vtok[:, kc * 4 + s, :],
                                       in_=accv[:, s, :])
                    q_mid.append(lambda kc=kc, outs=outs: mid_f(kc, outs))

                def mid_f(kc, outs):
                    outs2 = []
                    for g in range(G):
                        sq, vb = outs[g]
                        ssq = pssq.tile([1, KC], F32, tag="ssq", name="ssq")
                        nc.tensor.matmul(ssq, lhsT=ones_col_bf, rhs=sq,
                                         start=True, stop=True)
                        rms = tmp.tile([1, KC], F32, tag="rms", name="rms",
                                       bufs=3)
                        nc.scalar.activation(out=rms, in_=ssq, func=SQRT,
                                             scale=1.0 / D, bias=eps_t[:, :])
                        rinv = tmp.tile([1, KC], F32, tag="rinv", name="rinv",
                                        bufs=3)
                        nc.vector.reciprocal_approx_fast(out=rinv, in_=rms)
                        rinv_r = tmp.tile([1, KC], BF16, tag="rinvr",
                                          name="rinvr", bufs=5)
                        nc.vector.tensor_copy(out=rinv_r, in_=rinv)
                        outs2.append((vb, rinv_r))
                    q_bot.append(lambda kc=kc, outs2=outs2: bot_f(kc, outs2))

                def bot_f(kc, outs2):
                    for g in range(G):
                        vb, rinv_r = outs2[g]
                        bc = pbc.tile([128, KC], F32, tag="bc", name="bc")
                        nc.tensor.matmul(bc, lhsT=gk_sb, rhs=rinv_r,
                                         start=True, stop=True)
                        nc.vector.tensor_tensor(
                            out=ktn[g][:, kc * KC:(kc + 1) * KC],
                            in0=vb, in1=bc, op=MULT)

                q_top.append(top_f)
            while q_top or q_mid or q_bot:
                pop(q_top)
                pop(q_mid)
                pop(q_bot)

        # ------------- phase 2: Q projection (own 512 queries) ------------
        with ExitStack() as p2:
            tmp2 = p2.enter_context(tc.tile_pool(name="tmp2", bufs=3))
            pq = p2.enter_context(tc.tile_pool(name="pq", bufs=2, space="PSUM"))
            pssq2 = p2.enter_context(tc.tile_pool(name="pssq2", bufs=2,
                                                  space="PSUM"))
            pbcq = p2.enter_context(tc.tile_pool(name="pbcq", bufs=2,
                                                 space="PSUM"))
            for qc in range(2, 4):
                wq_sb = wqp.tile([128, ET * 128], BF16, tag="wq", name="wq")
                nc.sync.dma_start(out=wq_sb, in_=wq_d[:, qc, :])
                wq_tiles.append(wq_sb)
            q_top, q_mid, q_bot = [], [], []
            for qc in range(NH):
                if qc + 4 < NH:
                    wq_sb = wqp.tile([128, ET * 128], BF16, tag="wq",
                                     name="wq")
                    nc.sync.dma_start(out=wq_sb, in_=wq_d[:, qc + 4, :])
                    wq_tiles.append(wq_sb)
                pop(q_top)
                acc = pq.tile([128, QPC], F32, tag="qacc", name="qacc")
                for et in range(ET):
                    nc.tensor.matmul(
                        acc,
                        lhsT=wq_tiles[qc][:, et * 128:(et + 1) * 128],
                        rhs=xq_sb[:, et, :],
                        start=(et == 0), stop=(et == ET - 1))
                pop(q_mid)
                pop(q_bot)

                def top_f(qc=qc, acc=acc):
                    sq = tmp2.tile([128, QPC], BF16, tag="sq", name="qsq")
                    nc.scalar.activation(out=sq, in_=acc, func=SQUARE,
                                         bias=bq_sb[:, qc:qc + 1])
                    vb = tmp2.tile([128, QPC], F32, tag="vb", name="qvb",
                                   bufs=4)
                    nc.vector.tensor_scalar(
                        out=vb, in0=acc, scalar1=bq_sb[:, qc:qc + 1],
                        scalar2=None, op0=ADD)

                    def mid_f(qc=qc, sq=sq, vb=vb):
                        ssq = pssq2.tile([1, QPC], F32, tag="ssq",
                                         name="qssq")
                        nc.tensor.matmul(ssq, lhsT=ones_col_bf, rhs=sq,
                                         start=True, stop=True)
                        rms = tmp2.tile([1, QPC], F32, tag="rms", name="qrms")
                        nc.scalar.activation(out=rms, in_=ssq, func=SQRT,
                                             scale=1.0 / D, bias=eps_t[:, :])
                        rinv = tmp2.tile([1, QPC], F32, tag="rinv",
                                         name="qrinv", bufs=3)
                        nc.vector.reciprocal_approx_fast(out=rinv, in_=rms)
                        rinv_r = tmp2.tile([1, QPC], BF16, tag="rinvr",
                                           name="qrinvr", bufs=4)
                        nc.vector.tensor_copy(out=rinv_r, in_=rinv)

                        def bot_f(qc=qc, vb=vb, rinv_r=rinv_r):
                            bc = pbcq.tile([128, QPC], F32, tag="bc",
                                           name="qbc")
                            nc.tensor.matmul(bc, lhsT=gq_sb, rhs=rinv_r,
                                             start=True, stop=True)
                            nc.vector.tensor_tensor(out=qtn[:, qc, :],
                                                    in0=vb, in1=bc, op=MULT)
                        q_bot.append(bot_f)
                    q_mid.append(mid_f)
                q_top.append(top_f)
            while q_top or q_mid or q_bot:
                pop(q_top)
                pop(q_mid)
                pop(q_bot)

        # ---------- phase 3: causal attention + phase 4: out proj ---------
        with ExitStack() as p34:
            ptp = p34.enter_context(tc.tile_pool(name="pt", bufs=5))
            tmp3 = p34.enter_context(tc.tile_pool(name="tmp3", bufs=3))
            wop = p34.enter_context(tc.tile_pool(name="wos", bufs=16))
            osb = p34.enter_context(tc.tile_pool(name="osb", bufs=3))
            psc = p34.enter_context(tc.tile_pool(name="psc", bufs=2,
                                                 space="PSUM"))
            pden = p34.enter_context(tc.tile_pool(name="pden", bufs=2,
                                                  space="PSUM"))
            pcx = p34.enter_context(tc.tile_pool(name="pcx", bufs=2,
                                                 space="PSUM"))
            pend_exp = []
            pend_acc = []
            pendH = []

            def grp_offsets(grp):
                """[(kt, col offset in mega tile, width)] for a group."""
                out = []
                for pi, pair in enumerate(grp):
                    base = 512 * pi
                    for m, kt in enumerate(pair):
                        off = base + (0 if m == 0 else 512 - 32 * pair[0])
                        out.append((kt, off, 512 - 32 * kt))
                return out

            def post_exp(h, g, grp, sc, den, cx):
                used = 256 if len(grp) == 1 else 1024
                pt = ptp.tile([128, 2 * QPC], BF16, tag="pt", name="pt")
                nc.scalar.activation(out=pt[:, 0:used], in_=sc[:, 0:used],
                                     func=EXP, scale=SCALE)
                mem = grp_offsets(grp)
                for kt, off, w in mem:
                    nc.vector.tensor_tensor(out=pt[:, off:off + 32],
                                            in0=pt[:, off:off + 32],
                                            in1=dmask, op=MULT)

                def post_acc():
                    for kt, off, w in mem:
                        first = (kt == 0)
                        last = (kt == 8)
                        nc.tensor.matmul(den[:, 32 * kt:512],
                                         lhsT=ones_col_bf,
                                         rhs=pt[:, off:off + w],
                                         start=first, stop=last)
                        nc.tensor.matmul(
                            cx[:, 32 * kt:512],
                            lhsT=vtok[:, kt, g * 128:(g + 1) * 128],
                            rhs=pt[:, off:off + w],
                            start=first, stop=last)
                pend_acc.append(post_acc)

            def post_head(h, den, cx):
                rd = tmp3.tile([1, QPC], F32, tag="rd", name="rd")
                nc.vector.reciprocal_approx_fast(out=rd, in_=den)
                rd_bf = tmp3.tile([1, QPC], BF16, tag="rdbf", name="rdbf")
                nc.vector.tensor_copy(out=rd_bf, in_=rd)
                bc2 = psc.tile([128, 2 * QPC], F32, tag="sc", name="bc2")
                nc.tensor.matmul(bc2[:, 0:QPC], lhsT=ones_row_bf, rhs=rd_bf,
                                 start=True, stop=True)
                bc2s = tmp3.tile([128, QPC], F32, tag="bc2s", name="bc2s")
                nc.scalar.copy(out=bc2s, in_=bc2[:, 0:QPC])
                nc.vector.tensor_tensor(out=ctxt[:, h, :], in0=cx, in1=bc2s,
                                        op=MULT)

            # prefetch all Wo tiles while Sync is otherwise idle in phase 3
            wo_tiles = []
            for c2 in range(ET):
                wo_sb = wop.tile([128, ET * 128], BF16, tag="wo", name="wo")
                nc.sync.dma_start(out=wo_sb, in_=wo_d[:, c2, :])
                wo_tiles.append(wo_sb)

            for h in range(NH):
                g = h // GS
                den = pden.tile([1, QPC], F32, tag="den", name="den")
                cx = pcx.tile([128, QPC], F32, tag="cx", name="cx")
                for ti, grp in enumerate(SC_GROUPS):
                    if pend_acc:
                        pend_acc.pop(0)()
                    if pend_exp:
                        pend_exp.pop(0)()
                    if ti == 2 and pendH:
                        pendH.pop(0)()
                    sc = psc.tile([128, 2 * QPC], F32, tag="sc", name="sc")
                    for kt, off, w in grp_offsets(grp):
                        nc.tensor.matmul(
                            sc[:, off:off + w],
                            lhsT=ktn[g][:, kt * 128:(kt + 1) * 128],
                            rhs=qtn[:, h, 32 * kt:512],
                            start=True, stop=True)
                    pend_exp.append(
                        lambda h=h, g=g, grp=grp, sc=sc, den=den, cx=cx:
                        post_exp(h, g, grp, sc, den, cx))
                pendH.append(lambda h=h, den=den, cx=cx: post_head(h, den, cx))
            while pend_exp or pend_acc:
                if pend_acc:
                    pend_acc.pop(0)()
                if pend_exp:
                    pend_exp.pop(0)()
            while pendH:
                pendH.pop(0)()

            # ------------------------ phase 4: out proj -------------------
            pend4 = []
            for c2 in range(ET):
                while len(pend4) > 1:
                    pend4.pop(0)()
                wo_sb = wo_tiles[c2]
                acc = pcx.tile([128, QPC], F32, tag="cx", name="oacc")
                for ct in range(ET):
                    nc.tensor.matmul(acc,
                                     lhsT=wo_sb[:, ct * 128:(ct + 1) * 128],
                                     rhs=ctxt[:, ct, :],
                                     start=(ct == 0), stop=(ct == ET - 1))

                def post_o(c2=c2, acc=acc):
                    ot = osb.tile([128, QPC], F32, tag="ot", name="ot")
                    nc.vector.tensor_scalar(
                        out=ot, in0=acc, scalar1=bo_sb[:, c2:c2 + 1],
                        scalar2=None, op0=ADD)
                    nc.sync.dma_start(
                        out=out_d[c2 * 128:(c2 + 1) * 128, :], in_=ot)
                pend4.append(post_o)
            while pend4:
                pend4.pop(0)()
    nc.compile()
    return nc


# ---------------------------------------------------------------------------
# host-side sharding
# ---------------------------------------------------------------------------

def make_in_maps(cfg, inputs):
    B, S, E, D, G = cfg["B"], cfg["S"], cfg["E"], cfg["D"], cfg["G"]
    NH, ET, NKT, QPC, GS = derived(cfg)
    x = np.asarray(inputs["x"], np.float32)
    Wq = np.asarray(inputs["Wq"], np.float32)
    Wk = np.asarray(inputs["Wk"], np.float32)
    Wv = np.asarray(inputs["Wv"], np.float32)
    Wo = np.asarray(inputs["Wo"], np.float32)

    wqp = np.ascontiguousarray(
        Wq.reshape(ET, 128, NH, 128).transpose(1, 2, 0, 3)
        .reshape(128, NH, ET * 128).astype(BF))
    wop = np.ascontiguousarray(
        Wo.reshape(ET, 128, ET, 128).transpose(1, 2, 0, 3)
        .reshape(128, ET, ET * 128).astype(BF))
    wkp = np.ascontiguousarray(
        Wk.reshape(ET, 128, G * 128).transpose(1, 0, 2)
        .reshape(128, ET * G * 128).astype(BF))
    wvp = np.ascontiguousarray(
        Wv.reshape(ET, 128, G * 128).transpose(1, 0, 2)
        .reshape(128, ET * G * 128).astype(BF))

    shared = dict(
        WqP=wqp, WoP=wop, WkP=wkp, WvP=wvp,
        bq_t=np.ascontiguousarray(
            np.asarray(inputs["bq"], np.float32).reshape(NH, 128).T),
        bk_t=np.ascontiguousarray(
            np.asarray(inputs["bk"], np.float32).reshape(G, 128).T),
        bv_r=np.ascontiguousarray(
            np.asarray(inputs["bv"], np.float32).reshape(1, G * 128)
            .astype(BF)),
        bo_t=np.ascontiguousarray(
            np.asarray(inputs["bo"], np.float32).reshape(ET, 128).T),
        gq_r=np.ascontiguousarray(
            np.asarray(inputs["gamma_q"], np.float32).reshape(1, 128)
            .astype(BF)),
        gk_r=np.ascontiguousarray(
            np.asarray(inputs["gamma_k"], np.float32).reshape(1, 128)
            .astype(BF)),
    )
    xTb = [np.ascontiguousarray(x[b].T.astype(BF)) for b in range(B)]
    in_maps, perms = [], []
    for c in range(8):
        b, j = c // 4, c % 4
        kk = np.arange(128)[:, None]
        ii = np.arange(32)[None, :]
        dmask = (kk <= 4 * ii + j).astype(BF)
        m = dict(shared)
        m["xT"] = xTb[b]
        m["xq"] = np.ascontiguousarray(xTb[b][:, j::4])
        m["dmask"] = np.ascontiguousarray(dmask)
        in_maps.append(m)
        perms.append(j)
    return in_maps, perms


def assemble(cfg, results, perms):
    B, S, E = cfg["B"], cfg["S"], cfg["E"]
    out = np.empty((B, S, E), np.float32)
    for c in range(8):
        b, j = c // 4, perms[c]
        out[b, j::4, :] = results[c]["outT"].T
    return out


_CACHE = {}


def kernel(**inputs):
    cfg = full_cfg()
    if "nc" not in _CACHE:
        _CACHE["nc"] = build_program(cfg)
    nc = _CACHE["nc"]
    in_maps, perms = make_in_maps(cfg, inputs)
    res = run_bass_kernel_spmd(nc, in_maps, list(range(8)))
    return assemble(cfg, res.results, perms)
